# revision 2
# baseline (speedup 1.0000x reference)
"""Self-contained Trainium2 Bass kernel for nn_GCMCModel (GCMC GNN), v2.

Single fused launch:
  - bf16 embedding tables uploaded SHARDED (1/8 per core), AllGather'd on
    device over NeuronLink (replaces 8x replicated f32 upload = the old
    bottleneck: ~300MB over a ~60MB/s axon tunnel).
  - segment-sum aggregation via dma_gather + one-hot matmuls (as v1).
  - per-window inv-degree scaling + PE transpose, AllGather of the
    slot-major partial aggregates, per-batch gather, MLP, all on-core.
  - downloads only the final [1, B/8] per core.
"""

# ---- toolchain workarounds (this container's walrus supports only one
# sync-wait per instruction) -------------------------------------------------

def _apply_tile_fix():
    import concourse.mybir as mybir
    from concourse.tile import TileContext, ScopedClock
    if getattr(TileContext, "_drain_patched", False):
        return
    TileContext._drain_patched = True

    def _drain_and_barrier(self, tick_clock, wait_clock):
        nop = self.nc.sync.nop()
        wait_clock.add_sem_waits(nop.ins, ScopedClock({None: tick_clock.global_clock}))
        si = nop.ins.sync_info
        waits = list(si.on_wait) if si is not None else []
        if waits:
            si.on_wait = waits[:1]
        for w in waits[1:]:
            n2 = self.nc.sync.nop()
            n2.ins.sync_info = mybir.SyncInfo(on_wait=[w], on_update=[])
        self.nc.sync.drain()
        self.nc.all_engine_barrier()
        popped = self.nc._tile_sem_poison_stack.pop()
        assert popped is self._sem_poison
        self.nc.clear_and_free_semaphores(list(self.sems.allocated().values()))
        self.nc.all_engine_barrier()

    TileContext._drain_and_barrier = _drain_and_barrier


def _apply_bir_fix():
    import json as _json
    import concourse.bass_utils as _bu
    import concourse.bass2jax as _b2j
    if getattr(_bu, "_wait_split_patched", False):
        return
    _bu._wait_split_patched = True
    _orig = _bu.compile_bir_kernel
    _ctr = [0]

    def _split(bir_bytes):
        mod = _json.loads(bir_bytes)
        changed = False
        for fn in mod.get("functions", []):
            for blk in fn.get("blocks", []) or []:
                out = []
                for ins in blk.get("instructions", []):
                    si = ins.get("sync_info")
                    waits = (si or {}).get("on_wait") or []
                    if len(waits) > 1:
                        changed = True
                        for w in waits[:-1]:
                            _ctr[0] += 1
                            out.append({"debug": ins.get("debug", 0),
                                        "engine": ins["engine"], "ins": [],
                                        "name": f"{ins['name']}-ws{_ctr[0]}",
                                        "opcode": "NoOp", "outs": [],
                                        "sync_info": {"on_update": [],
                                                      "on_wait": [w]}})
                        si["on_wait"] = [waits[-1]]
                    out.append(ins)
                blk["instructions"] = out
        return _json.dumps(mod).encode() if changed else bir_bytes

    def _patched(bir_json, tmpdir, neff_name="file.neff"):
        if isinstance(bir_json, str):
            bir_json = bir_json.encode()
        return _orig(_split(bir_json), tmpdir, neff_name)

    _bu.compile_bir_kernel = _patched
    _b2j.compile_bir_kernel = _patched

_apply_tile_fix()
_apply_bir_fix()

import time as _time
import numpy as np
import concourse.bacc as bacc
import concourse.mybir as mybir
from concourse.tile import TileContext
from concourse import bass_utils

EXEC_SECONDS = []

N_CORES = 8
P = 128
GG = 16          # tiles per dma_gather group
UHALF = 65024    # user table gather split point (must be even)

# packed-pairs table geometry (two 64-d rows per 128-wide packed row, one
# leading zero pair-row, padded to a multiple of 8 for sharding)
N_USER, N_ITEM = 100000, 50000
NUT = -((-(N_USER // 2 + 1)) // 8) * 8     # 50008 packed user rows
NIT = -((-(N_ITEM // 2 + 1)) // 8) * 8     # 25008 packed item rows
NUS, NIS = NUT // 8, NIT // 8              # per-core shard rows
A_ROWS = UHALF // 2 + 1                    # 32513: packed rows for users < UHALF
B_BASE = A_ROWS - 1                        # 32512: B slice starts here
B_ROWS = NUT - B_BASE                      # 17496
PADB = (N_USER // 2 + 1) - B_BASE          # 17489: zero pad row inside B slice


def _build_side(n_slots, slot_of_edge, val_idx, val_par, val_half, n_cores):
    """Bin edges by (core, window, half) into a STRUCTURALLY UNIFORM tile grid:
    every core gets Wc windows x (KA half-A tiles + KB half-B tiles). Tile t:
    window = t // K, half = 0 if t % K < KA else 1, acc offset = window*128.
    Pad slots use value-idx 0 (a zero row), so they contribute nothing."""
    w = (slot_of_edge >> 7).astype(np.int64)
    n_win = (n_slots + 127) // 128
    Wc = (n_win + n_cores - 1) // n_cores
    r = (slot_of_edge & 127).astype(np.int64)
    halves = val_half if val_half is not None else np.zeros(len(w), np.int8)

    key = w * 2 + halves
    order = np.argsort(key, kind="stable")
    key_s = key[order]
    starts = np.searchsorted(key_s, np.arange(n_win * 2))
    ends = np.searchsorted(key_s, np.arange(n_win * 2) + 1)
    cnt = (ends - starts).reshape(n_win, 2)
    KA = max(1, int(np.ceil(cnt[:, 0].max() / P))) if cnt[:, 0].max() else 1
    KB = int(np.ceil(cnt[:, 1].max() / P)) if val_half is not None and cnt[:, 1].max() else 0
    K = KA + KB
    T = Wc * K

    half_tile = np.zeros(T, np.int8)
    off_tile = np.zeros(T, np.int64)
    for t in range(T):
        off_tile[t] = (t // K) * 128
        half_tile[t] = 0 if (t % K) < KA else 1

    per_core = []
    for c in range(n_cores):
        r_grid = np.zeros((P, T), np.float16)
        p_grid = np.zeros((P, T), np.uint8)
        idx_grid = np.zeros((T, P), np.int16)
        for li in range(Wc):
            win = li * n_cores + c
            if win >= n_win:
                continue
            for h, base_t, cap in ((0, li * K, KA), (1, li * K + KA, KB)):
                g0, g1 = starts[win * 2 + h], ends[win * 2 + h]
                eids = order[g0:g1]
                assert len(eids) <= cap * P
                for j, s0 in enumerate(range(0, len(eids), P)):
                    seg = eids[s0:s0 + P]
                    n = len(seg)
                    t = base_t + j
                    r_grid[:n, t] = r[seg]
                    p_grid[:n, t] = val_par[seg]
                    idx_grid[t, :n] = val_idx[seg]
        # half-B tiles gather from utab[B_BASE:]; their pad lanes (idx 0)
        # must point at a zero row INSIDE the B slice.
        for t in range(T):
            if half_tile[t] == 1:
                col = idx_grid[t]
                col[col == 0] = PADB
        per_core.append(dict(r_grid=r_grid, p_grid=p_grid, idx_grid=idx_grid,
                             half_tile=half_tile, off_tile=off_tile))
    return per_core, Wc, T


def _wrap16(idx_flat):
    """[N] int16 -> [16, N/16] (16-lane wrap; device replicates x8)."""
    n = len(idx_flat)
    assert n % 16 == 0
    return np.ascontiguousarray(idx_flat.reshape(n // 16, 16).T)




def _layout(TT, Wc_u, Wc_i, Bc):
    """Column layout (u8) of the consolidated per-core blob [128, CB]."""
    def al(x, a):
        return -(-x // a) * a
    off = {}
    c = 0
    off["rgp"] = (c, TT); c = al(c + TT, 2)
    off["ix"] = (c, 2 * TT); c += 2 * TT
    off["brow"] = (c, 2 * (Bc // 16)); c += 2 * (Bc // 16)
    off["bpar"] = (c, 32); c = al(c + 32, 4)
    off["idg_u"] = (c, 4 * Wc_u); c += 4 * Wc_u
    off["idg_i"] = (c, 4 * Wc_i); c += 4 * Wc_i
    off["W1"] = (c, 1024); c += 1024
    off["W2"] = (c, 256); c += 256
    off["WuWi"] = (c, 256); c += 256
    off["sm"] = (c, 12); c += 12
    return off, al(c, 4)

def build_fused(T_u, off_u, half_u, T_i, off_i, half_i, Wc_u, Wc_i, Bc,
                no_gather=False, no_compute=False):
    nc = bacc.Bacc()
    dt = mybir.dt
    TT = T_u + T_i
    GU = Bc // 128   # per-b gather groups per side
    RG8 = list(range(8))

    L, CB = _layout(TT, Wc_u, Wc_i, Bc)
    tsh = nc.dram_tensor("tsh", [NUS + NIS, 128], dt.float16, kind="ExternalInput")
    blob = nc.dram_tensor("blob", [128, CB], dt.uint8, kind="ExternalInput")
    out = nc.dram_tensor("out", [1, Bc], dt.float32, kind="ExternalOutput")

    CH = 512
    with TileContext(nc) as tc:
        with tc.tile_pool(name="st", bufs=1) as st, \
             tc.tile_pool(name="dram", bufs=1, space="DRAM") as dram:
            # ---- DRAM scratch + table AllGather ----
            ish_b = dram.tile([NIS, 128], dt.float16)
            ush_b = dram.tile([NUS, 128], dt.float16)
            itab = dram.tile([NIT, 128], dt.float16)
            utab = dram.tile([NUT, 128], dt.float16)
            accT_u_l = dram.tile([Wc_u * 128, 64], dt.float32)
            accT_i_l = dram.tile([Wc_i * 128, 64], dt.float32)
            accT_u_f = dram.tile([Wc_u * 128 * 8, 64], dt.float32)
            accT_i_f = dram.tile([Wc_i * 128 * 8, 64], dt.float32)
            nc.gpsimd.dma_start(ush_b[:, :], tsh[0:NUS, :])
            nc.gpsimd.dma_start(ish_b[:, :], tsh[NUS:NUS + NIS, :])
            nc.gpsimd.collective_compute(
                "AllGather", mybir.AluOpType.bypass, replica_groups=[RG8],
                ins=[ish_b.opt()], outs=[itab.opt()])
            nc.gpsimd.collective_compute(
                "AllGather", mybir.AluOpType.bypass, replica_groups=[RG8],
                ins=[ush_b.opt()], outs=[utab.opt()])

            # ---- SBUF staging (from consolidated blob) ----
            ixt = st.tile([128, TT * 8], dt.int16)
            c0, _ = L["ix"]
            for r in range(8):
                for k in range(8):
                    nc.sync.dma_start(
                        out=ixt[16 * r:16 * r + 16, k * TT:(k + 1) * TT],
                        in_=blob[16 * k:16 * k + 16,
                                 c0:c0 + 2 * TT].bitcast(dt.int16))
            c0, _ = L["rgp"]
            rgp = st.tile([P, TT], dt.uint8)
            nc.sync.dma_start(out=rgp[:], in_=blob[:, c0:c0 + TT])
            rgc = st.tile([P, TT], dt.uint8)
            pm = st.tile([P, TT], dt.uint8)
            nc.vector.tensor_scalar(out=rgc[:], in0=rgp[:], scalar1=127,
                                    scalar2=None,
                                    op0=mybir.AluOpType.bitwise_and)
            nc.vector.tensor_scalar(out=pm[:], in0=rgp[:], scalar1=127,
                                    scalar2=None,
                                    op0=mybir.AluOpType.is_gt)
            c0, _ = L["idg_u"]
            idg_ut = st.tile([P, Wc_u], dt.float32)
            nc.sync.dma_start(out=idg_ut[:],
                              in_=blob[:, c0:c0 + 4 * Wc_u].bitcast(dt.float32))
            c0, _ = L["idg_i"]
            idg_it = st.tile([P, Wc_i], dt.float32)
            nc.sync.dma_start(out=idg_it[:],
                              in_=blob[:, c0:c0 + 4 * Wc_i].bitcast(dt.float32))

            iotaGG = st.tile([128, GG, 128], dt.uint8)
            nc.gpsimd.iota(iotaGG[:, :, :], [[0, GG], [1, 128]],
                           channel_multiplier=0,
                           allow_small_or_imprecise_dtypes=True)
            iotaI = st.tile([128, 128], dt.int32)
            nc.gpsimd.iota(iotaI[:], [[1, 128]], channel_multiplier=0)
            iota32 = st.tile([128, 128], dt.float32)
            nc.vector.tensor_copy(iota32[:], iotaI[:])
            parI = st.tile([128, 1], dt.int32)
            nc.gpsimd.iota(parI[:], [[0, 1]], channel_multiplier=1)
            parF = st.tile([128, 1], dt.float32)
            nc.vector.tensor_copy(parF[:], parI[:])
            ident = st.tile([128, 128], dt.float32)
            nc.vector.tensor_tensor(out=ident[:], in0=iota32[:],
                                    in1=parF[:].to_broadcast([128, 128]),
                                    op=mybir.AluOpType.is_equal)

            acc_u = st.tile([64, Wc_u * 128], dt.float32)
            acc_i = st.tile([64, Wc_i * 128], dt.float32)
            nc.vector.memset(acc_u[:], 0.0)
            nc.vector.memset(acc_i[:], 0.0)

            # ---- phase A: aggregation ----
            with tc.tile_pool(name="g", bufs=3) as gp, \
                 tc.tile_pool(name="w", bufs=8) as wp, \
                 tc.tile_pool(name="psA", bufs=3, space="PSUM") as pp:
                for side in ("u", "i"):
                    T = T_u if side == "u" else T_i
                    base = 0 if side == "u" else T_u
                    offs = off_u if side == "u" else off_i
                    halves = half_u if side == "u" else half_i
                    acc = acc_u if side == "u" else acc_i
                    # gather groups: runs of tiles sharing a source table
                    groups = []
                    t0 = 0
                    while t0 < T:
                        t1 = t0
                        while t1 < T and t1 - t0 < GG and halves[t1] == halves[t0]:
                            t1 += 1
                        groups.append((t0, t1))
                        t0 = t1
                    oh_of = {}
                    vs_of = {}
                    for (a, b) in groups:
                        nt = b - a
                        if side == "u":
                            tab_ap = itab[:, :]
                        else:
                            tab_ap = (utab[0:A_ROWS, :] if halves[a] == 0
                                      else utab[B_BASE:NUT, :])
                        vp = gp.tile([P, GG, 128], dt.float16, tag="vp")
                        if no_gather:
                            nc.vector.memset(vp[:, :nt, :], 0.0)
                        else:
                            nc.gpsimd.dma_gather(
                                out_ap=vp[:, :nt, :], in_ap=tab_ap,
                                idxs_ap=ixt[:, (base + a) * 8:(base + b) * 8],
                                num_idxs=nt * 128, num_idxs_reg=nt * 128,
                                elem_size=128, single_packet=False)
                        oh = gp.tile([P, GG, 128], dt.float16, tag="oh")
                        nc.vector.tensor_tensor(
                            out=oh[:, :nt, :],
                            in0=rgc[:, base + a:base + b].to_broadcast([P, nt, 128]),
                            in1=iotaGG[:, :nt, :],
                            op=mybir.AluOpType.is_equal)
                        vs = gp.tile([P, GG, 64], dt.float16, tag="vs")
                        nc.vector.select(
                            out=vs[:, :nt, :],
                            mask=pm[:, base + a:base + b].to_broadcast([P, nt, 64]),
                            on_true=vp[:, :nt, 64:128],
                            on_false=vp[:, :nt, 0:64])
                        for t in range(a, b):
                            oh_of[t] = (oh, t - a)
                            vs_of[t] = (vs, t - a)
                    t = 0
                    while t < T and not no_compute:
                        o = int(offs[t])
                        K_w = 1
                        while t + K_w < T and int(offs[t + K_w]) == o:
                            K_w += 1
                        ps = pp.tile([64, 128], dt.float32, tag="ps")
                        for j in range(K_w):
                            tt = t + j
                            oh, oi = oh_of[tt]
                            vs, vi = vs_of[tt]
                            nc.tensor.matmul(ps[:], lhsT=vs[:, vi, :],
                                             rhs=oh[:, oi, :],
                                             start=(j == 0), stop=(j == K_w - 1))
                        nc.vector.tensor_add(
                            out=acc[:, o:o + 128], in0=acc[:, o:o + 128], in1=ps[:])
                        t += K_w

                # ---- inv-deg scale + transpose to slot-major, local DRAM ----
                for side in ("u", "i"):
                    Wc = Wc_u if side == "u" else Wc_i
                    acc = acc_u if side == "u" else acc_i
                    idg = idg_ut if side == "u" else idg_it
                    accT = accT_u_l if side == "u" else accT_i_l
                    for li in range(Wc):
                        pT = pp.tile([128, 64], dt.float32, tag="pt")
                        nc.tensor.transpose(pT[:], acc[:, li * 128:(li + 1) * 128],
                                            ident[0:64, 0:64])
                        sT = wp.tile([128, 64], dt.float32, tag="st")
                        nc.vector.tensor_tensor(
                            out=sT[:], in0=pT[:],
                            in1=idg[:, li:li + 1].to_broadcast([128, 64]),
                            op=mybir.AluOpType.mult)
                        nc.sync.dma_start(out=accT[li * 128:(li + 1) * 128, :],
                                          in_=sT[:])
                nc.gpsimd.collective_compute(
                    "AllGather", mybir.AluOpType.bypass, replica_groups=[RG8],
                    ins=[accT_u_l.opt()], outs=[accT_u_f.opt()])
                nc.gpsimd.collective_compute(
                    "AllGather", mybir.AluOpType.bypass, replica_groups=[RG8],
                    ins=[accT_i_l.opt()], outs=[accT_i_f.opt()])

            # ---- phase B: per-b gather + MLP ----
            with tc.tile_pool(name="w2", bufs=2) as wp2, \
                 tc.tile_pool(name="psB", bufs=1, space="PSUM") as pp2:
                gu = st.tile([64, Bc], dt.float32)
                gi = st.tile([64, Bc], dt.float32)
                cbr, _ = L["brow"]
                for side in ("u", "i"):
                    brows = (0, 16) if side == "u" else (16, 32)
                    accF = accT_u_f if side == "u" else accT_i_f
                    dst = gi if side == "u" else gu  # user-side agg -> gcn_item_h
                    browt = st.tile([128, Bc // 16], dt.int16,
                                    tag=f"brow{side}")
                    for k in range(8):
                        nc.sync.dma_start(
                            out=browt[16 * k:16 * k + 16, :],
                            in_=blob[brows[0]:brows[1],
                                     cbr:cbr + 2 * (Bc // 16)].bitcast(dt.int16))
                    gg = st.tile([128, GU, 64], dt.float32, tag=f"gg{side}")
                    nc.gpsimd.dma_gather(
                        out_ap=gg[:, :, :], in_ap=accF[:, :],
                        idxs_ap=browt[:, :], num_idxs=Bc, num_idxs_reg=Bc,
                        elem_size=64, single_packet=False)
                    for k in range(GU):
                        pG = pp2.tile([64, 128], dt.float32, tag="pg")
                        nc.tensor.transpose(pG[:], gg[:, k, :], ident[:, :])
                        nc.vector.tensor_copy(dst[:, k * 128:(k + 1) * 128], pG[:])

                # ---- device-side ue/ie gather from AllGather'd tables ----
                ident16 = st.tile([128, 128], dt.float16)
                nc.vector.tensor_copy(ident16[:], ident[:])
                cpar, _ = L["bpar"]
                pu_t = st.tile([128, 16], dt.uint8)
                pi_t = st.tile([128, 16], dt.uint8)
                nc.sync.dma_start(out=pu_t[:], in_=blob[:, cpar:cpar + 16])
                nc.sync.dma_start(out=pi_t[:], in_=blob[:, cpar + 16:cpar + 32])
                bw = 2 * (Bc // 16)
                eidx = {}
                for nm, rr in (("ie", (32, 48)), ("ueA", (48, 64)),
                               ("ueB", (64, 80))):
                    ei_t = st.tile([128, Bc // 16], dt.int16, tag=f"eix{nm}")
                    for k in range(8):
                        nc.sync.dma_start(
                            out=ei_t[16 * k:16 * k + 16, :],
                            in_=blob[rr[0]:rr[1], cbr:cbr + bw].bitcast(dt.int16))
                    eidx[nm] = ei_t
                g_ie = st.tile([128, GU, 128], dt.float16)
                nc.gpsimd.dma_gather(
                    out_ap=g_ie[:, :, :], in_ap=itab[:, :], idxs_ap=eidx["ie"][:, :],
                    num_idxs=Bc, num_idxs_reg=Bc, elem_size=128, single_packet=False)
                g_uA = st.tile([128, GU, 128], dt.float16)
                nc.gpsimd.dma_gather(
                    out_ap=g_uA[:, :, :], in_ap=utab[0:A_ROWS, :],
                    idxs_ap=eidx["ueA"][:, :],
                    num_idxs=Bc, num_idxs_reg=Bc, elem_size=128, single_packet=False)
                g_uB = st.tile([128, GU, 128], dt.float16)
                nc.gpsimd.dma_gather(
                    out_ap=g_uB[:, :, :], in_ap=utab[B_BASE:NUT, :],
                    idxs_ap=eidx["ueB"][:, :],
                    num_idxs=Bc, num_idxs_reg=Bc, elem_size=128, single_packet=False)
                g_ue = st.tile([128, GU, 128], dt.float16)
                nc.vector.tensor_add(g_ue[:, :, :], g_uA[:, :, :], g_uB[:, :, :])
                se_u = st.tile([128, GU, 64], dt.float16)
                nc.vector.select(
                    out=se_u[:, :, :],
                    mask=pu_t[:, :].to_broadcast([128, GU, 64]),
                    on_true=g_ue[:, :, 64:128], on_false=g_ue[:, :, 0:64])
                se_i = st.tile([128, GU, 64], dt.float16)
                nc.vector.select(
                    out=se_i[:, :, :],
                    mask=pi_t[:, :].to_broadcast([128, GU, 64]),
                    on_true=g_ie[:, :, 64:128], on_false=g_ie[:, :, 0:64])
                ue_f = st.tile([64, Bc], dt.float32)
                ie_f = st.tile([64, Bc], dt.float32)
                for se, dst in ((se_u, ue_f), (se_i, ie_f)):
                    for k in range(GU):
                        pE = pp2.tile([64, 128], dt.float16, tag="pe")
                        nc.tensor.matmul(pE[:], lhsT=se[:, k, :], rhs=ident16[:],
                                         is_transpose=True, start=True, stop=True)
                        nc.vector.tensor_copy(dst[:, k * 128:(k + 1) * 128], pE[:])

                cW1, _ = L["W1"]
                t_W1 = st.tile([64, 4 * 128], dt.float32)
                for k in range(4):
                    r0 = (k % 2) * 64
                    cc = cW1 + (k // 2) * 512
                    nc.sync.dma_start(out=t_W1[:, 128 * k:128 * k + 128],
                                      in_=blob[r0:r0 + 64,
                                               cc:cc + 512].bitcast(dt.float32))
                cW2, _ = L["W2"]
                t_W2 = st.tile([128, 64], dt.float32)
                nc.sync.dma_start(out=t_W2[:],
                                  in_=blob[:, cW2:cW2 + 256].bitcast(dt.float32))
                cWw, _ = L["WuWi"]
                t_Wu = st.tile([64, 64], dt.float32)
                t_Wi = st.tile([64, 64], dt.float32)
                nc.sync.dma_start(out=t_Wu[:],
                                  in_=blob[0:64, cWw:cWw + 256].bitcast(dt.float32))
                nc.sync.dma_start(out=t_Wi[:],
                                  in_=blob[64:128, cWw:cWw + 256].bitcast(dt.float32))
                csm, _ = L["sm"]
                t_b1 = st.tile([128, 1], dt.float32)
                nc.sync.dma_start(out=t_b1[:],
                                  in_=blob[:, csm:csm + 4].bitcast(dt.float32))
                t_bu = st.tile([64, 1], dt.float32)
                t_bi = st.tile([64, 1], dt.float32)
                nc.sync.dma_start(out=t_bu[:],
                                  in_=blob[0:64, csm + 4:csm + 8].bitcast(dt.float32))
                nc.sync.dma_start(out=t_bi[:],
                                  in_=blob[64:128, csm + 4:csm + 8].bitcast(dt.float32))
                t_b2 = st.tile([64, 1], dt.float32)
                t_W3 = st.tile([64, 1], dt.float32)
                nc.sync.dma_start(out=t_b2[:],
                                  in_=blob[0:64, csm + 8:csm + 12].bitcast(dt.float32))
                nc.sync.dma_start(out=t_W3[:],
                                  in_=blob[64:128, csm + 8:csm + 12].bitcast(dt.float32))

                guo = st.tile([64, Bc], dt.float32)
                gio = st.tile([64, Bc], dt.float32)
                h1 = st.tile([128, Bc], dt.float32)
                h2 = st.tile([64, Bc], dt.float32)
                res = st.tile([1, Bc], dt.float32)
                for c0 in range(0, Bc, CH):
                    c1 = min(c0 + CH, Bc)
                    p1 = pp2.tile([64, CH], dt.float32, tag="p1")
                    nc.tensor.matmul(p1[:, :c1 - c0], lhsT=t_Wu[:],
                                     rhs=gu[:, c0:c1], start=True, stop=True)
                    nc.scalar.activation(guo[:, c0:c1], p1[:, :c1 - c0],
                                         mybir.ActivationFunctionType.Relu,
                                         bias=t_bu[:], scale=1.0)
                    p2 = pp2.tile([64, CH], dt.float32, tag="p2")
                    nc.tensor.matmul(p2[:, :c1 - c0], lhsT=t_Wi[:],
                                     rhs=gi[:, c0:c1], start=True, stop=True)
                    nc.scalar.activation(gio[:, c0:c1], p2[:, :c1 - c0],
                                         mybir.ActivationFunctionType.Relu,
                                         bias=t_bi[:], scale=1.0)
                    prods = []
                    for (x_, y_) in ((ue_f, ie_f), (ue_f, gio), (guo, ie_f),
                                     (guo, gio)):
                        pr = wp2.tile([64, CH], dt.float32,
                                      tag=f"pr{len(prods)}")
                        nc.vector.tensor_mul(pr[:, :c1 - c0], x_[:, c0:c1],
                                             y_[:, c0:c1])
                        prods.append(pr)
                    p3 = pp2.tile([128, CH], dt.float32, tag="p3")
                    for k in range(4):
                        nc.tensor.matmul(p3[:, :c1 - c0],
                                         lhsT=t_W1[:, 128 * k:128 * k + 128],
                                         rhs=prods[k][:, :c1 - c0],
                                         start=(k == 0), stop=(k == 3))
                    nc.scalar.activation(h1[:, c0:c1], p3[:, :c1 - c0],
                                         mybir.ActivationFunctionType.Tanh,
                                         bias=t_b1[:], scale=1.0)
                    p4 = pp2.tile([64, CH], dt.float32, tag="p4")
                    nc.tensor.matmul(p4[:, :c1 - c0], lhsT=t_W2[:],
                                     rhs=h1[:, c0:c1], start=True, stop=True)
                    nc.scalar.activation(h2[:, c0:c1], p4[:, :c1 - c0],
                                         mybir.ActivationFunctionType.Tanh,
                                         bias=t_b2[:], scale=1.0)
                    p5 = pp2.tile([1, CH], dt.float32, tag="p5")
                    nc.tensor.matmul(p5[:, :c1 - c0], lhsT=t_W3[:],
                                     rhs=h2[:, c0:c1], start=True, stop=True)
                    nc.vector.tensor_copy(res[:, c0:c1], p5[:, :c1 - c0])
                nc.sync.dma_start(out=out[:, :], in_=res[:])
    nc.compile()
    return nc


def host_prep(user_table, item_table, Wu, bu, Wi, bi, W1, b1, W2, b2, W3, b3,
              user_bias, item_bias, user_id, item_id, edge_user, edge_item):
    user_table = np.asarray(user_table, np.float32)
    item_table = np.asarray(item_table, np.float32)
    user_id = np.asarray(user_id).astype(np.int64)
    item_id = np.asarray(item_id).astype(np.int64)
    eu = np.asarray(edge_user).astype(np.int64)
    ei = np.asarray(edge_item).astype(np.int64)
    n_user, D = user_table.shape
    n_item = item_table.shape[0]
    assert (n_user, n_item) == (N_USER, N_ITEM)
    B = len(user_id)
    Bc = B // N_CORES

    # ---- host prep ----
    uu = np.unique(user_id)
    ui = np.unique(item_id)
    pos_u = np.full(n_user, -1, np.int64); pos_u[uu] = np.arange(len(uu))
    pos_i = np.full(n_item, -1, np.int64); pos_i[ui] = np.arange(len(ui))

    deg_u_full = np.bincount(eu, minlength=n_user).astype(np.float32) + 1.0
    deg_i_full = np.bincount(ei, minlength=n_item).astype(np.float32) + 1.0

    # user-side: slots over users, values = item pair-rows
    su = pos_u[eu]
    mu = su >= 0
    vi_u = ((ei[mu] >> 1) + 1).astype(np.int16)
    vp_u = (ei[mu] & 1).astype(np.float16)
    side_u, Wc_u, T_u = _build_side(len(uu), su[mu], vi_u, vp_u, None, N_CORES)

    # item-side: slots over items, values = user pair-rows (A/B halves for
    # the int16 gather-index limit)
    si = pos_i[ei]
    mi = si >= 0
    uh = (eu[mi] >= UHALF).astype(np.int8)
    relu_ = eu[mi] - uh.astype(np.int64) * UHALF
    vi_i = ((relu_ >> 1) + 1).astype(np.int16)
    # half-B indices are relative to utab[B_BASE:]: global packed row is
    # 1 + (eu >> 1); B-relative = that - B_BASE
    gb = eu[mi] >> 1
    vi_i = np.where(uh == 1, (gb + 1 - B_BASE), vi_i).astype(np.int16)
    vp_i = (eu[mi] & 1).astype(np.float16)
    side_i, Wc_i, T_i = _build_side(len(ui), si[mi], vi_i, vp_i, uh, N_CORES)

    # packed-pairs tables, bf16, padded to NCORES shards
    def pack_shards(tb, tot_rows):
        n = tb.shape[0]
        pad2 = (-n) % 2
        tbp = np.vstack([np.zeros((2, 64), np.float32), tb,
                         np.zeros((pad2, 64), np.float32)])
        packed = tbp.reshape(-1, 128)
        full = np.zeros((tot_rows, 128), np.float32)
        full[:packed.shape[0]] = packed
        return np.ascontiguousarray(full).astype(np.float16)
    itab_sh = pack_shards(item_table, NIT).reshape(N_CORES, NIS, 128)
    utab_sh = pack_shards(user_table, NUT).reshape(N_CORES, NUS, 128)

    off_u0 = side_u[0]["off_tile"]; half_u0 = side_u[0]["half_tile"]
    off_i0 = side_i[0]["off_tile"]; half_i0 = side_i[0]["half_tile"]
    for c in range(1, N_CORES):
        assert (side_u[c]["off_tile"] == off_u0).all()
        assert (side_i[c]["off_tile"] == off_i0).all()
        assert (side_u[c]["half_tile"] == half_u0).all()
        assert (side_i[c]["half_tile"] == half_i0).all()

    n_win_u = (len(uu) + 127) // 128
    n_win_i = (len(ui) + 127) // 128

    # per-b rows into the AllGather'd slot-major layout:
    # row(slot) = (w%8)*Wc*128 + (w//8)*128 + (slot&127),  w = slot>>7
    def slot_rows(slots, Wc):
        w = slots >> 7
        return ((w % N_CORES) * (Wc * 128) + (w // N_CORES) * 128
                + (slots & 127)).astype(np.int16)
    brow_u_all = slot_rows(pos_u[user_id], Wc_u)
    brow_i_all = slot_rows(pos_i[item_id], Wc_i)

    # inv-degree per slot, laid out per core as [128, Wc]
    def inv_deg_grid(c, Wc, n_win, uniq, deg_full):
        g = np.ones((128, Wc), np.float32)
        for li in range(Wc):
            w = li * N_CORES + c
            if w >= n_win:
                continue
            s0 = w * 128
            s1 = min(s0 + 128, len(uniq))
            if s1 > s0:
                g[:s1 - s0, li] = 1.0 / deg_full[uniq[s0:s1]]
        return g

    bias_b = (np.float32(np.asarray(b3).reshape(-1)[0])
              + np.asarray(user_bias)[user_id, 0]
              + np.asarray(item_bias)[item_id, 0]).astype(np.float32)

    TT = T_u + T_i
    L, CB = _layout(TT, Wc_u, Wc_i, Bc)
    W1f = np.asarray(W1, np.float32)
    W2f = np.asarray(W2, np.float32)
    Wuf = np.asarray(Wu, np.float32)
    Wif = np.asarray(Wi, np.float32)
    sm = np.zeros((128, 3), np.float32)
    sm[:, 0] = np.asarray(b1, np.float32).reshape(-1)
    sm[0:64, 1] = np.asarray(bu, np.float32).reshape(-1)
    sm[64:128, 1] = np.asarray(bi, np.float32).reshape(-1)
    sm[0:64, 2] = np.asarray(b2, np.float32).reshape(-1)
    sm[64:128, 2] = W3f = np.asarray(W3, np.float32).reshape(-1)

    in_maps = []
    for c in range(N_CORES):
        du, di = side_u[c], side_i[c]
        sl = slice(c * Bc, (c + 1) * Bc)
        blob = np.zeros((128, CB), np.uint8)
        rp = (np.concatenate([du["r_grid"], di["r_grid"]], axis=1)
              .astype(np.uint8)
              + 128 * np.concatenate([du["p_grid"], di["p_grid"]], axis=1))
        c0, _ = L["rgp"]; blob[:, c0:c0 + TT] = rp
        ix_w = _wrap16(np.concatenate([du["idx_grid"].reshape(-1),
                                       di["idx_grid"].reshape(-1)]))
        c0, _ = L["ix"]
        for k in range(8):
            blob[16 * k:16 * k + 16, c0:c0 + 2 * TT] = \
                ix_w[:, k * TT:(k + 1) * TT].copy().view(np.uint8)
        c0, _ = L["brow"]
        bw = 2 * (Bc // 16)
        blob[0:16, c0:c0 + bw] = _wrap16(brow_u_all[sl]).view(np.uint8)
        blob[16:32, c0:c0 + bw] = _wrap16(brow_i_all[sl]).view(np.uint8)
        uid_c = user_id[sl]; iid_c = item_id[sl]
        ie_idx = (1 + (iid_c >> 1)).astype(np.int16)
        ueA_idx = np.where(uid_c < UHALF, 1 + (uid_c >> 1), 0).astype(np.int16)
        ueB_idx = np.where(uid_c >= UHALF, 1 + (uid_c >> 1) - B_BASE,
                           PADB).astype(np.int16)
        blob[32:48, c0:c0 + bw] = _wrap16(ie_idx).view(np.uint8)
        blob[48:64, c0:c0 + bw] = _wrap16(ueA_idx).view(np.uint8)
        blob[64:80, c0:c0 + bw] = _wrap16(ueB_idx).view(np.uint8)
        c0, _ = L["bpar"]
        blob[:, c0:c0 + 16] = (uid_c & 1).astype(np.uint8).reshape(16, 128).T
        blob[:, c0 + 16:c0 + 32] = (iid_c & 1).astype(np.uint8).reshape(16, 128).T
        c0, _ = L["idg_u"]
        blob[:, c0:c0 + 4 * Wc_u] = \
            inv_deg_grid(c, Wc_u, n_win_u, uu, deg_u_full).view(np.uint8)
        c0, _ = L["idg_i"]
        blob[:, c0:c0 + 4 * Wc_i] = \
            inv_deg_grid(c, Wc_i, n_win_i, ui, deg_i_full).view(np.uint8)
        c0, _ = L["W1"]
        for k in range(4):
            r0 = (k % 2) * 64
            cc = c0 + (k // 2) * 512
            blob[r0:r0 + 64, cc:cc + 512] = \
                np.ascontiguousarray(W1f[64 * k:64 * k + 64, :]).view(np.uint8)
        c0, _ = L["W2"]; blob[:, c0:c0 + 256] = W2f.view(np.uint8)
        c0, _ = L["WuWi"]
        blob[0:64, c0:c0 + 256] = Wuf.view(np.uint8)
        blob[64:128, c0:c0 + 256] = Wif.view(np.uint8)
        c0, _ = L["sm"]; blob[:, c0:c0 + 12] = sm.view(np.uint8)
        in_maps.append(dict(
            tsh=np.concatenate([utab_sh[c], itab_sh[c]], axis=0),
            blob=blob,
        ))
    return dict(T_u=T_u, off_u=off_u0, half_u=half_u0, T_i=T_i, off_i=off_i0,
                half_i=half_i0, Wc_u=Wc_u, Wc_i=Wc_i, Bc=Bc,
                bias_b=bias_b), in_maps


def kernel(**inputs):
    EXEC_SECONDS.clear()
    args, in_maps = host_prep(**inputs)
    nc1 = build_fused(args["T_u"], args["off_u"], args["half_u"], args["T_i"],
                      args["off_i"], args["half_i"], args["Wc_u"], args["Wc_i"],
                      args["Bc"])
    _t0 = _time.perf_counter()
    res = bass_utils.run_bass_kernel_spmd(nc1, in_maps, core_ids=list(range(N_CORES)))
    EXEC_SECONDS.append(_time.perf_counter() - _t0)
    out = np.concatenate([res.results[c]["out"][0] for c in range(N_CORES)])
    return (out + args["bias_b"]).astype(np.float32)


# revision 3
# speedup vs baseline: 2.3048x; 2.3048x over previous
"""Self-contained Trainium2 Bass kernel for nn_GCMCModel (GCMC GNN), v2.

Single fused launch:
  - bf16 embedding tables uploaded SHARDED (1/8 per core), AllGather'd on
    device over NeuronLink (replaces 8x replicated f32 upload = the old
    bottleneck: ~300MB over a ~60MB/s axon tunnel).
  - segment-sum aggregation via dma_gather + one-hot matmuls (as v1).
  - per-window inv-degree scaling + PE transpose, AllGather of the
    slot-major partial aggregates, per-batch gather, MLP, all on-core.
  - downloads only the final [1, B/8] per core.
"""

# ---- toolchain workarounds (this container's walrus supports only one
# sync-wait per instruction) -------------------------------------------------

def _apply_tile_fix():
    import concourse.mybir as mybir
    from concourse.tile import TileContext, ScopedClock
    if getattr(TileContext, "_drain_patched", False):
        return
    TileContext._drain_patched = True

    def _drain_and_barrier(self, tick_clock, wait_clock):
        nop = self.nc.sync.nop()
        wait_clock.add_sem_waits(nop.ins, ScopedClock({None: tick_clock.global_clock}))
        si = nop.ins.sync_info
        waits = list(si.on_wait) if si is not None else []
        if waits:
            si.on_wait = waits[:1]
        for w in waits[1:]:
            n2 = self.nc.sync.nop()
            n2.ins.sync_info = mybir.SyncInfo(on_wait=[w], on_update=[])
        self.nc.sync.drain()
        self.nc.all_engine_barrier()
        popped = self.nc._tile_sem_poison_stack.pop()
        assert popped is self._sem_poison
        self.nc.clear_and_free_semaphores(list(self.sems.allocated().values()))
        self.nc.all_engine_barrier()

    TileContext._drain_and_barrier = _drain_and_barrier


def _apply_bir_fix():
    import json as _json
    import concourse.bass_utils as _bu
    import concourse.bass2jax as _b2j
    if getattr(_bu, "_wait_split_patched", False):
        return
    _bu._wait_split_patched = True
    _orig = _bu.compile_bir_kernel
    _ctr = [0]

    def _split(bir_bytes):
        mod = _json.loads(bir_bytes)
        changed = False
        for fn in mod.get("functions", []):
            for blk in fn.get("blocks", []) or []:
                out = []
                for ins in blk.get("instructions", []):
                    si = ins.get("sync_info")
                    waits = (si or {}).get("on_wait") or []
                    if len(waits) > 1:
                        changed = True
                        for w in waits[:-1]:
                            _ctr[0] += 1
                            out.append({"debug": ins.get("debug", 0),
                                        "engine": ins["engine"], "ins": [],
                                        "name": f"{ins['name']}-ws{_ctr[0]}",
                                        "opcode": "NoOp", "outs": [],
                                        "sync_info": {"on_update": [],
                                                      "on_wait": [w]}})
                        si["on_wait"] = [waits[-1]]
                    out.append(ins)
                blk["instructions"] = out
        return _json.dumps(mod).encode() if changed else bir_bytes

    def _patched(bir_json, tmpdir, neff_name="file.neff"):
        if isinstance(bir_json, str):
            bir_json = bir_json.encode()
        return _orig(_split(bir_json), tmpdir, neff_name)

    _bu.compile_bir_kernel = _patched
    _b2j.compile_bir_kernel = _patched

_apply_tile_fix()
_apply_bir_fix()

import time as _time
import numpy as np
import concourse.bacc as bacc
import concourse.mybir as mybir
from concourse.tile import TileContext
from concourse import bass_utils

EXEC_SECONDS = []

N_CORES = 8
P = 128
GG = 32          # tiles per dma_gather group
UHALF = 65024    # user table gather split point (must be even)

# packed-pairs table geometry (two 64-d rows per 128-wide packed row, one
# leading zero pair-row, padded to a multiple of 8 for sharding)
N_USER, N_ITEM = 100000, 50000
NUT = -((-(N_USER // 2 + 1)) // 8) * 8     # 50008 packed user rows
NIT = -((-(N_ITEM // 2 + 1)) // 8) * 8     # 25008 packed item rows
NUS, NIS = NUT // 8, NIT // 8              # per-core shard rows
A_ROWS = UHALF // 2 + 1                    # 32513: packed rows for users < UHALF
B_BASE = A_ROWS - 1                        # 32512: B slice starts here
B_ROWS = NUT - B_BASE                      # 17496
PADB = (N_USER // 2 + 1) - B_BASE          # 17489: zero pad row inside B slice


def _build_side(n_slots, slot_of_edge, val_idx, val_par, val_half, n_cores):
    """Bin edges by (core, window, half) into a STRUCTURALLY UNIFORM tile grid:
    every core gets Wc windows x (KA half-A tiles + KB half-B tiles). Tile t:
    window = t // K, half = 0 if t % K < KA else 1, acc offset = window*128.
    Pad slots use value-idx 0 (a zero row), so they contribute nothing."""
    w = (slot_of_edge >> 7).astype(np.int64)
    n_win = (n_slots + 127) // 128
    Wc = (n_win + n_cores - 1) // n_cores
    r = (slot_of_edge & 127).astype(np.int64)
    halves = val_half if val_half is not None else np.zeros(len(w), np.int8)

    key = w * 2 + halves
    order = np.argsort(key, kind="stable")
    key_s = key[order]
    starts = np.searchsorted(key_s, np.arange(n_win * 2))
    ends = np.searchsorted(key_s, np.arange(n_win * 2) + 1)
    cnt = (ends - starts).reshape(n_win, 2)
    KA = max(1, int(np.ceil(cnt[:, 0].max() / P))) if cnt[:, 0].max() else 1
    KB = int(np.ceil(cnt[:, 1].max() / P)) if val_half is not None and cnt[:, 1].max() else 0
    K = KA + KB
    T = Wc * K

    half_tile = np.zeros(T, np.int8)
    off_tile = np.zeros(T, np.int64)
    for t in range(T):
        off_tile[t] = (t // K) * 128
        half_tile[t] = 0 if (t % K) < KA else 1

    per_core = []
    for c in range(n_cores):
        r_grid = np.zeros((P, T), np.float16)
        p_grid = np.zeros((P, T), np.uint8)
        idx_grid = np.zeros((T, P), np.int16)
        for li in range(Wc):
            win = li * n_cores + c
            if win >= n_win:
                continue
            for h, base_t, cap in ((0, li * K, KA), (1, li * K + KA, KB)):
                g0, g1 = starts[win * 2 + h], ends[win * 2 + h]
                eids = order[g0:g1]
                assert len(eids) <= cap * P
                for j, s0 in enumerate(range(0, len(eids), P)):
                    seg = eids[s0:s0 + P]
                    n = len(seg)
                    t = base_t + j
                    r_grid[:n, t] = r[seg]
                    p_grid[:n, t] = val_par[seg]
                    idx_grid[t, :n] = val_idx[seg]
        # half-B tiles gather from utab[B_BASE:]; their pad lanes (idx 0)
        # must point at a zero row INSIDE the B slice.
        for t in range(T):
            if half_tile[t] == 1:
                col = idx_grid[t]
                col[col == 0] = PADB
        per_core.append(dict(r_grid=r_grid, p_grid=p_grid, idx_grid=idx_grid,
                             half_tile=half_tile, off_tile=off_tile))
    return per_core, Wc, T


def _wrap16(idx_flat):
    """[N] int16 -> [16, N/16] (16-lane wrap; device replicates x8)."""
    n = len(idx_flat)
    assert n % 16 == 0
    return np.ascontiguousarray(idx_flat.reshape(n // 16, 16).T)




def _layout(TT, Wc_u, Wc_i, Bc):
    """Column layout (u8) of the consolidated per-core blob [128, CB]."""
    def al(x, a):
        return -(-x // a) * a
    off = {}
    c = 0
    off["rgp"] = (c, TT); c = al(c + TT, 2)
    off["ix"] = (c, 2 * TT); c += 2 * TT
    off["brow"] = (c, 2 * (Bc // 16)); c += 2 * (Bc // 16)
    off["bpar"] = (c, 32); c = al(c + 32, 4)
    off["idg_u"] = (c, 4 * Wc_u); c += 4 * Wc_u
    off["idg_i"] = (c, 4 * Wc_i); c += 4 * Wc_i
    off["W1"] = (c, 1024); c += 1024
    off["W2"] = (c, 256); c += 256
    off["WuWi"] = (c, 256); c += 256
    off["sm"] = (c, 12); c += 12
    return off, al(c, 4)

def build_fused(T_u, off_u, half_u, T_i, off_i, half_i, Wc_u, Wc_i, Bc,
                no_gather=False, no_compute=False):
    nc = bacc.Bacc()
    dt = mybir.dt
    TT = T_u + T_i
    GU = Bc // 128   # per-b gather groups per side
    RG8 = list(range(8))

    L, CB = _layout(TT, Wc_u, Wc_i, Bc)
    tsh = nc.dram_tensor("tsh", [NUS + NIS, 128], dt.float16, kind="ExternalInput")
    blob = nc.dram_tensor("blob", [128, CB], dt.uint8, kind="ExternalInput")
    out = nc.dram_tensor("out", [1, Bc], dt.float32, kind="ExternalOutput")

    CH = 512
    with TileContext(nc) as tc:
        with tc.tile_pool(name="st", bufs=1) as st, \
             tc.tile_pool(name="dram", bufs=1, space="DRAM") as dram:
            # ---- DRAM scratch + table AllGather ----
            ish_b = dram.tile([NIS, 128], dt.float16)
            ush_b = dram.tile([NUS, 128], dt.float16)
            itab = dram.tile([NIT, 128], dt.float16)
            utab = dram.tile([NUT, 128], dt.float16)
            accT_u_l = dram.tile([Wc_u * 128, 64], dt.float32)
            accT_i_l = dram.tile([Wc_i * 128, 64], dt.float32)
            accT_u_f = dram.tile([Wc_u * 128 * 8, 64], dt.float32)
            accT_i_f = dram.tile([Wc_i * 128 * 8, 64], dt.float32)
            nc.gpsimd.dma_start(ush_b[:, :], tsh[0:NUS, :])
            nc.gpsimd.dma_start(ish_b[:, :], tsh[NUS:NUS + NIS, :])
            nc.gpsimd.collective_compute(
                "AllGather", mybir.AluOpType.bypass, replica_groups=[RG8],
                ins=[ish_b.opt()], outs=[itab.opt()])
            nc.gpsimd.collective_compute(
                "AllGather", mybir.AluOpType.bypass, replica_groups=[RG8],
                ins=[ush_b.opt()], outs=[utab.opt()])

            # ---- SBUF staging (from consolidated blob) ----
            ixt = st.tile([128, TT * 8], dt.int16)
            c0, _ = L["ix"]
            for r in range(8):
                for k in range(8):
                    nc.sync.dma_start(
                        out=ixt[16 * r:16 * r + 16, k * TT:(k + 1) * TT],
                        in_=blob[16 * k:16 * k + 16,
                                 c0:c0 + 2 * TT].bitcast(dt.int16))
            c0, _ = L["rgp"]
            rgp = st.tile([P, TT], dt.uint8)
            nc.sync.dma_start(out=rgp[:], in_=blob[:, c0:c0 + TT])
            rgc = st.tile([P, TT], dt.uint8)
            pm = st.tile([P, TT], dt.uint8)
            nc.vector.tensor_scalar(out=rgc[:], in0=rgp[:], scalar1=127,
                                    scalar2=None,
                                    op0=mybir.AluOpType.bitwise_and)
            nc.vector.tensor_scalar(out=pm[:], in0=rgp[:], scalar1=127,
                                    scalar2=None,
                                    op0=mybir.AluOpType.is_gt)
            c0, _ = L["idg_u"]
            idg_ut = st.tile([P, Wc_u], dt.float32)
            nc.sync.dma_start(out=idg_ut[:],
                              in_=blob[:, c0:c0 + 4 * Wc_u].bitcast(dt.float32))
            c0, _ = L["idg_i"]
            idg_it = st.tile([P, Wc_i], dt.float32)
            nc.sync.dma_start(out=idg_it[:],
                              in_=blob[:, c0:c0 + 4 * Wc_i].bitcast(dt.float32))

            iotaGG = st.tile([128, GG, 128], dt.uint8)
            nc.gpsimd.iota(iotaGG[:, :, :], [[0, GG], [1, 128]],
                           channel_multiplier=0,
                           allow_small_or_imprecise_dtypes=True)
            iotaI = st.tile([128, 128], dt.int32)
            nc.gpsimd.iota(iotaI[:], [[1, 128]], channel_multiplier=0)
            iota32 = st.tile([128, 128], dt.float32)
            nc.vector.tensor_copy(iota32[:], iotaI[:])
            parI = st.tile([128, 1], dt.int32)
            nc.gpsimd.iota(parI[:], [[0, 1]], channel_multiplier=1)
            parF = st.tile([128, 1], dt.float32)
            nc.vector.tensor_copy(parF[:], parI[:])
            ident = st.tile([128, 128], dt.float32)
            nc.vector.tensor_tensor(out=ident[:], in0=iota32[:],
                                    in1=parF[:].to_broadcast([128, 128]),
                                    op=mybir.AluOpType.is_equal)

            acc_u = st.tile([64, Wc_u * 128], dt.float32)
            acc_i = st.tile([64, Wc_i * 128], dt.float32)
            nc.vector.memset(acc_u[:], 0.0)
            nc.vector.memset(acc_i[:], 0.0)

            # ---- phase A: aggregation ----
            with tc.tile_pool(name="g", bufs=2) as gp, \
                 tc.tile_pool(name="w", bufs=8) as wp, \
                 tc.tile_pool(name="psA", bufs=3, space="PSUM") as pp:
                for side in ("u", "i"):
                    T = T_u if side == "u" else T_i
                    base = 0 if side == "u" else T_u
                    offs = off_u if side == "u" else off_i
                    halves = half_u if side == "u" else half_i
                    acc = acc_u if side == "u" else acc_i
                    # gather groups: runs of tiles sharing a source table
                    groups = []
                    t0 = 0
                    while t0 < T:
                        t1 = t0
                        while t1 < T and t1 - t0 < GG and halves[t1] == halves[t0]:
                            t1 += 1
                        groups.append((t0, t1))
                        t0 = t1
                    oh_of = {}
                    vs_of = {}
                    for (a, b) in groups:
                        nt = b - a
                        if side == "u":
                            tab_ap = itab[:, :]
                        else:
                            tab_ap = (utab[0:A_ROWS, :] if halves[a] == 0
                                      else utab[B_BASE:NUT, :])
                        vp = gp.tile([P, GG, 128], dt.float16, tag="vp")
                        if no_gather:
                            nc.vector.memset(vp[:, :nt, :], 0.0)
                        else:
                            nc.gpsimd.dma_gather(
                                out_ap=vp[:, :nt, :], in_ap=tab_ap,
                                idxs_ap=ixt[:, (base + a) * 8:(base + b) * 8],
                                num_idxs=nt * 128, num_idxs_reg=nt * 128,
                                elem_size=128, single_packet=False)
                        oh = gp.tile([P, GG, 128], dt.float16, tag="oh")
                        nc.vector.tensor_tensor(
                            out=oh[:, :nt, :],
                            in0=rgc[:, base + a:base + b].to_broadcast([P, nt, 128]),
                            in1=iotaGG[:, :nt, :],
                            op=mybir.AluOpType.is_equal)
                        vs = gp.tile([P, GG, 64], dt.float16, tag="vs")
                        nc.vector.select(
                            out=vs[:, :nt, :],
                            mask=pm[:, base + a:base + b].to_broadcast([P, nt, 64]),
                            on_true=vp[:, :nt, 64:128],
                            on_false=vp[:, :nt, 0:64])
                        for t in range(a, b):
                            oh_of[t] = (oh, t - a)
                            vs_of[t] = (vs, t - a)
                    t = 0
                    while t < T and not no_compute:
                        o = int(offs[t])
                        K_w = 1
                        while t + K_w < T and int(offs[t + K_w]) == o:
                            K_w += 1
                        ps = pp.tile([64, 128], dt.float32, tag="ps")
                        for j in range(K_w):
                            tt = t + j
                            oh, oi = oh_of[tt]
                            vs, vi = vs_of[tt]
                            nc.tensor.matmul(ps[:], lhsT=vs[:, vi, :],
                                             rhs=oh[:, oi, :],
                                             start=(j == 0), stop=(j == K_w - 1))
                        nc.vector.tensor_add(
                            out=acc[:, o:o + 128], in0=acc[:, o:o + 128], in1=ps[:])
                        t += K_w

                # ---- device-side ue/ie gather from AllGather'd tables ----
                ident16 = st.tile([128, 128], dt.float16)
                nc.vector.tensor_copy(ident16[:], ident[:])
                cpar, _ = L["bpar"]
                pu_t = st.tile([128, 16], dt.uint8)
                pi_t = st.tile([128, 16], dt.uint8)
                nc.sync.dma_start(out=pu_t[:], in_=blob[:, cpar:cpar + 16])
                nc.sync.dma_start(out=pi_t[:], in_=blob[:, cpar + 16:cpar + 32])
                cbr, _ = L["brow"]
                bw = 2 * (Bc // 16)
                eidx = {}
                for nm, rr in (("ie", (32, 48)), ("ueA", (48, 64)),
                               ("ueB", (64, 80))):
                    ei_t = st.tile([128, Bc // 16], dt.int16, tag=f"eix{nm}")
                    for k in range(8):
                        nc.sync.dma_start(
                            out=ei_t[16 * k:16 * k + 16, :],
                            in_=blob[rr[0]:rr[1], cbr:cbr + bw].bitcast(dt.int16))
                    eidx[nm] = ei_t
                g_ie = st.tile([128, GU, 128], dt.float16)
                nc.gpsimd.dma_gather(
                    out_ap=g_ie[:, :, :], in_ap=itab[:, :], idxs_ap=eidx["ie"][:, :],
                    num_idxs=Bc, num_idxs_reg=Bc, elem_size=128, single_packet=False)
                g_uA = st.tile([128, GU, 128], dt.float16)
                nc.gpsimd.dma_gather(
                    out_ap=g_uA[:, :, :], in_ap=utab[0:A_ROWS, :],
                    idxs_ap=eidx["ueA"][:, :],
                    num_idxs=Bc, num_idxs_reg=Bc, elem_size=128, single_packet=False)
                g_uB = st.tile([128, GU, 128], dt.float16)
                nc.gpsimd.dma_gather(
                    out_ap=g_uB[:, :, :], in_ap=utab[B_BASE:NUT, :],
                    idxs_ap=eidx["ueB"][:, :],
                    num_idxs=Bc, num_idxs_reg=Bc, elem_size=128, single_packet=False)
                g_ue = st.tile([128, GU, 128], dt.float16)
                nc.vector.tensor_add(g_ue[:, :, :], g_uA[:, :, :], g_uB[:, :, :])
                se_u = st.tile([128, GU, 64], dt.float16)
                nc.vector.select(
                    out=se_u[:, :, :],
                    mask=pu_t[:, :].to_broadcast([128, GU, 64]),
                    on_true=g_ue[:, :, 64:128], on_false=g_ue[:, :, 0:64])
                se_i = st.tile([128, GU, 64], dt.float16)
                nc.vector.select(
                    out=se_i[:, :, :],
                    mask=pi_t[:, :].to_broadcast([128, GU, 64]),
                    on_true=g_ie[:, :, 64:128], on_false=g_ie[:, :, 0:64])

                # ---- inv-deg scale + transpose to slot-major, local DRAM ----
                for side in ("u", "i"):
                    Wc = Wc_u if side == "u" else Wc_i
                    acc = acc_u if side == "u" else acc_i
                    idg = idg_ut if side == "u" else idg_it
                    accT = accT_u_l if side == "u" else accT_i_l
                    for li in range(Wc):
                        pT = pp.tile([128, 64], dt.float32, tag="pt")
                        nc.tensor.transpose(pT[:], acc[:, li * 128:(li + 1) * 128],
                                            ident[0:64, 0:64])
                        sT = wp.tile([128, 64], dt.float32, tag="st")
                        nc.vector.tensor_tensor(
                            out=sT[:], in0=pT[:],
                            in1=idg[:, li:li + 1].to_broadcast([128, 64]),
                            op=mybir.AluOpType.mult)
                        nc.sync.dma_start(out=accT[li * 128:(li + 1) * 128, :],
                                          in_=sT[:])
                nc.gpsimd.collective_compute(
                    "AllGather", mybir.AluOpType.bypass, replica_groups=[RG8],
                    ins=[accT_u_l.opt()], outs=[accT_u_f.opt()])
                nc.gpsimd.collective_compute(
                    "AllGather", mybir.AluOpType.bypass, replica_groups=[RG8],
                    ins=[accT_i_l.opt()], outs=[accT_i_f.opt()])

            # ---- phase B: per-b gather + MLP ----
            with tc.tile_pool(name="w2", bufs=2) as wp2, \
                 tc.tile_pool(name="psB", bufs=1, space="PSUM") as pp2:
                gu = st.tile([64, Bc], dt.float32)
                gi = st.tile([64, Bc], dt.float32)
                cbr, _ = L["brow"]
                for side in ("u", "i"):
                    brows = (0, 16) if side == "u" else (16, 32)
                    accF = accT_u_f if side == "u" else accT_i_f
                    dst = gi if side == "u" else gu  # user-side agg -> gcn_item_h
                    browt = st.tile([128, Bc // 16], dt.int16,
                                    tag=f"brow{side}")
                    for k in range(8):
                        nc.sync.dma_start(
                            out=browt[16 * k:16 * k + 16, :],
                            in_=blob[brows[0]:brows[1],
                                     cbr:cbr + 2 * (Bc // 16)].bitcast(dt.int16))
                    gg = st.tile([128, GU, 64], dt.float32, tag=f"gg{side}")
                    nc.gpsimd.dma_gather(
                        out_ap=gg[:, :, :], in_ap=accF[:, :],
                        idxs_ap=browt[:, :], num_idxs=Bc, num_idxs_reg=Bc,
                        elem_size=64, single_packet=False)
                    for k in range(GU):
                        pG = pp2.tile([64, 128], dt.float32, tag="pg")
                        nc.tensor.transpose(pG[:], gg[:, k, :], ident[:, :])
                        nc.vector.tensor_copy(dst[:, k * 128:(k + 1) * 128], pG[:])

                ue_f = st.tile([64, Bc], dt.float32)
                ie_f = st.tile([64, Bc], dt.float32)
                for se, dst in ((se_u, ue_f), (se_i, ie_f)):
                    for k in range(GU):
                        pE = pp2.tile([64, 128], dt.float16, tag="pe")
                        nc.tensor.matmul(pE[:], lhsT=se[:, k, :], rhs=ident16[:],
                                         is_transpose=True, start=True, stop=True)
                        nc.vector.tensor_copy(dst[:, k * 128:(k + 1) * 128], pE[:])

                cW1, _ = L["W1"]
                t_W1 = st.tile([64, 4 * 128], dt.float32)
                for k in range(4):
                    r0 = (k % 2) * 64
                    cc = cW1 + (k // 2) * 512
                    nc.sync.dma_start(out=t_W1[:, 128 * k:128 * k + 128],
                                      in_=blob[r0:r0 + 64,
                                               cc:cc + 512].bitcast(dt.float32))
                cW2, _ = L["W2"]
                t_W2 = st.tile([128, 64], dt.float32)
                nc.sync.dma_start(out=t_W2[:],
                                  in_=blob[:, cW2:cW2 + 256].bitcast(dt.float32))
                cWw, _ = L["WuWi"]
                t_Wu = st.tile([64, 64], dt.float32)
                t_Wi = st.tile([64, 64], dt.float32)
                nc.sync.dma_start(out=t_Wu[:],
                                  in_=blob[0:64, cWw:cWw + 256].bitcast(dt.float32))
                nc.sync.dma_start(out=t_Wi[:],
                                  in_=blob[64:128, cWw:cWw + 256].bitcast(dt.float32))
                csm, _ = L["sm"]
                t_b1 = st.tile([128, 1], dt.float32)
                nc.sync.dma_start(out=t_b1[:],
                                  in_=blob[:, csm:csm + 4].bitcast(dt.float32))
                t_bu = st.tile([64, 1], dt.float32)
                t_bi = st.tile([64, 1], dt.float32)
                nc.sync.dma_start(out=t_bu[:],
                                  in_=blob[0:64, csm + 4:csm + 8].bitcast(dt.float32))
                nc.sync.dma_start(out=t_bi[:],
                                  in_=blob[64:128, csm + 4:csm + 8].bitcast(dt.float32))
                t_b2 = st.tile([64, 1], dt.float32)
                t_W3 = st.tile([64, 1], dt.float32)
                nc.sync.dma_start(out=t_b2[:],
                                  in_=blob[0:64, csm + 8:csm + 12].bitcast(dt.float32))
                nc.sync.dma_start(out=t_W3[:],
                                  in_=blob[64:128, csm + 8:csm + 12].bitcast(dt.float32))

                guo = st.tile([64, Bc], dt.float32)
                gio = st.tile([64, Bc], dt.float32)
                h1 = st.tile([128, Bc], dt.float32)
                h2 = st.tile([64, Bc], dt.float32)
                res = st.tile([1, Bc], dt.float32)
                for c0 in range(0, Bc, CH):
                    c1 = min(c0 + CH, Bc)
                    p1 = pp2.tile([64, CH], dt.float32, tag="p1")
                    nc.tensor.matmul(p1[:, :c1 - c0], lhsT=t_Wu[:],
                                     rhs=gu[:, c0:c1], start=True, stop=True)
                    nc.scalar.activation(guo[:, c0:c1], p1[:, :c1 - c0],
                                         mybir.ActivationFunctionType.Relu,
                                         bias=t_bu[:], scale=1.0)
                    p2 = pp2.tile([64, CH], dt.float32, tag="p2")
                    nc.tensor.matmul(p2[:, :c1 - c0], lhsT=t_Wi[:],
                                     rhs=gi[:, c0:c1], start=True, stop=True)
                    nc.scalar.activation(gio[:, c0:c1], p2[:, :c1 - c0],
                                         mybir.ActivationFunctionType.Relu,
                                         bias=t_bi[:], scale=1.0)
                    prods = []
                    for (x_, y_) in ((ue_f, ie_f), (ue_f, gio), (guo, ie_f),
                                     (guo, gio)):
                        pr = wp2.tile([64, CH], dt.float32,
                                      tag=f"pr{len(prods)}")
                        nc.vector.tensor_mul(pr[:, :c1 - c0], x_[:, c0:c1],
                                             y_[:, c0:c1])
                        prods.append(pr)
                    p3 = pp2.tile([128, CH], dt.float32, tag="p3")
                    for k in range(4):
                        nc.tensor.matmul(p3[:, :c1 - c0],
                                         lhsT=t_W1[:, 128 * k:128 * k + 128],
                                         rhs=prods[k][:, :c1 - c0],
                                         start=(k == 0), stop=(k == 3))
                    nc.scalar.activation(h1[:, c0:c1], p3[:, :c1 - c0],
                                         mybir.ActivationFunctionType.Tanh,
                                         bias=t_b1[:], scale=1.0)
                    p4 = pp2.tile([64, CH], dt.float32, tag="p4")
                    nc.tensor.matmul(p4[:, :c1 - c0], lhsT=t_W2[:],
                                     rhs=h1[:, c0:c1], start=True, stop=True)
                    nc.scalar.activation(h2[:, c0:c1], p4[:, :c1 - c0],
                                         mybir.ActivationFunctionType.Tanh,
                                         bias=t_b2[:], scale=1.0)
                    p5 = pp2.tile([1, CH], dt.float32, tag="p5")
                    nc.tensor.matmul(p5[:, :c1 - c0], lhsT=t_W3[:],
                                     rhs=h2[:, c0:c1], start=True, stop=True)
                    nc.vector.tensor_copy(res[:, c0:c1], p5[:, :c1 - c0])
                nc.sync.dma_start(out=out[:, :], in_=res[:])
    nc.compile()
    return nc


def host_prep(user_table, item_table, Wu, bu, Wi, bi, W1, b1, W2, b2, W3, b3,
              user_bias, item_bias, user_id, item_id, edge_user, edge_item):
    user_table = np.asarray(user_table, np.float32)
    item_table = np.asarray(item_table, np.float32)
    user_id = np.asarray(user_id).astype(np.int64)
    item_id = np.asarray(item_id).astype(np.int64)
    eu = np.asarray(edge_user).astype(np.int64)
    ei = np.asarray(edge_item).astype(np.int64)
    n_user, D = user_table.shape
    n_item = item_table.shape[0]
    assert (n_user, n_item) == (N_USER, N_ITEM)
    B = len(user_id)
    Bc = B // N_CORES

    # ---- host prep ----
    uu = np.unique(user_id)
    ui = np.unique(item_id)
    pos_u = np.full(n_user, -1, np.int64); pos_u[uu] = np.arange(len(uu))
    pos_i = np.full(n_item, -1, np.int64); pos_i[ui] = np.arange(len(ui))

    deg_u_full = np.bincount(eu, minlength=n_user).astype(np.float32) + 1.0
    deg_i_full = np.bincount(ei, minlength=n_item).astype(np.float32) + 1.0

    # user-side: slots over users, values = item pair-rows
    su = pos_u[eu]
    mu = su >= 0
    vi_u = ((ei[mu] >> 1) + 1).astype(np.int16)
    vp_u = (ei[mu] & 1).astype(np.float16)
    side_u, Wc_u, T_u = _build_side(len(uu), su[mu], vi_u, vp_u, None, N_CORES)

    # item-side: slots over items, values = user pair-rows (A/B halves for
    # the int16 gather-index limit)
    si = pos_i[ei]
    mi = si >= 0
    uh = (eu[mi] >= UHALF).astype(np.int8)
    relu_ = eu[mi] - uh.astype(np.int64) * UHALF
    vi_i = ((relu_ >> 1) + 1).astype(np.int16)
    # half-B indices are relative to utab[B_BASE:]: global packed row is
    # 1 + (eu >> 1); B-relative = that - B_BASE
    gb = eu[mi] >> 1
    vi_i = np.where(uh == 1, (gb + 1 - B_BASE), vi_i).astype(np.int16)
    vp_i = (eu[mi] & 1).astype(np.float16)
    side_i, Wc_i, T_i = _build_side(len(ui), si[mi], vi_i, vp_i, uh, N_CORES)

    # packed-pairs tables, bf16, padded to NCORES shards
    def pack_shards(tb, tot_rows):
        n = tb.shape[0]
        pad2 = (-n) % 2
        tbp = np.vstack([np.zeros((2, 64), np.float32), tb,
                         np.zeros((pad2, 64), np.float32)])
        packed = tbp.reshape(-1, 128)
        full = np.zeros((tot_rows, 128), np.float32)
        full[:packed.shape[0]] = packed
        return np.ascontiguousarray(full).astype(np.float16)
    itab_sh = pack_shards(item_table, NIT).reshape(N_CORES, NIS, 128)
    utab_sh = pack_shards(user_table, NUT).reshape(N_CORES, NUS, 128)

    off_u0 = side_u[0]["off_tile"]; half_u0 = side_u[0]["half_tile"]
    off_i0 = side_i[0]["off_tile"]; half_i0 = side_i[0]["half_tile"]
    for c in range(1, N_CORES):
        assert (side_u[c]["off_tile"] == off_u0).all()
        assert (side_i[c]["off_tile"] == off_i0).all()
        assert (side_u[c]["half_tile"] == half_u0).all()
        assert (side_i[c]["half_tile"] == half_i0).all()

    n_win_u = (len(uu) + 127) // 128
    n_win_i = (len(ui) + 127) // 128

    # per-b rows into the AllGather'd slot-major layout:
    # row(slot) = (w%8)*Wc*128 + (w//8)*128 + (slot&127),  w = slot>>7
    def slot_rows(slots, Wc):
        w = slots >> 7
        return ((w % N_CORES) * (Wc * 128) + (w // N_CORES) * 128
                + (slots & 127)).astype(np.int16)
    brow_u_all = slot_rows(pos_u[user_id], Wc_u)
    brow_i_all = slot_rows(pos_i[item_id], Wc_i)

    # inv-degree per slot, laid out per core as [128, Wc]
    def inv_deg_grid(c, Wc, n_win, uniq, deg_full):
        g = np.ones((128, Wc), np.float32)
        for li in range(Wc):
            w = li * N_CORES + c
            if w >= n_win:
                continue
            s0 = w * 128
            s1 = min(s0 + 128, len(uniq))
            if s1 > s0:
                g[:s1 - s0, li] = 1.0 / deg_full[uniq[s0:s1]]
        return g

    bias_b = (np.float32(np.asarray(b3).reshape(-1)[0])
              + np.asarray(user_bias)[user_id, 0]
              + np.asarray(item_bias)[item_id, 0]).astype(np.float32)

    TT = T_u + T_i
    L, CB = _layout(TT, Wc_u, Wc_i, Bc)
    W1f = np.asarray(W1, np.float32)
    W2f = np.asarray(W2, np.float32)
    Wuf = np.asarray(Wu, np.float32)
    Wif = np.asarray(Wi, np.float32)
    sm = np.zeros((128, 3), np.float32)
    sm[:, 0] = np.asarray(b1, np.float32).reshape(-1)
    sm[0:64, 1] = np.asarray(bu, np.float32).reshape(-1)
    sm[64:128, 1] = np.asarray(bi, np.float32).reshape(-1)
    sm[0:64, 2] = np.asarray(b2, np.float32).reshape(-1)
    sm[64:128, 2] = W3f = np.asarray(W3, np.float32).reshape(-1)

    in_maps = []
    for c in range(N_CORES):
        du, di = side_u[c], side_i[c]
        sl = slice(c * Bc, (c + 1) * Bc)
        blob = np.zeros((128, CB), np.uint8)
        rp = (np.concatenate([du["r_grid"], di["r_grid"]], axis=1)
              .astype(np.uint8)
              + 128 * np.concatenate([du["p_grid"], di["p_grid"]], axis=1))
        c0, _ = L["rgp"]; blob[:, c0:c0 + TT] = rp
        ix_w = _wrap16(np.concatenate([du["idx_grid"].reshape(-1),
                                       di["idx_grid"].reshape(-1)]))
        c0, _ = L["ix"]
        for k in range(8):
            blob[16 * k:16 * k + 16, c0:c0 + 2 * TT] = \
                ix_w[:, k * TT:(k + 1) * TT].copy().view(np.uint8)
        c0, _ = L["brow"]
        bw = 2 * (Bc // 16)
        blob[0:16, c0:c0 + bw] = _wrap16(brow_u_all[sl]).view(np.uint8)
        blob[16:32, c0:c0 + bw] = _wrap16(brow_i_all[sl]).view(np.uint8)
        uid_c = user_id[sl]; iid_c = item_id[sl]
        ie_idx = (1 + (iid_c >> 1)).astype(np.int16)
        ueA_idx = np.where(uid_c < UHALF, 1 + (uid_c >> 1), 0).astype(np.int16)
        ueB_idx = np.where(uid_c >= UHALF, 1 + (uid_c >> 1) - B_BASE,
                           PADB).astype(np.int16)
        blob[32:48, c0:c0 + bw] = _wrap16(ie_idx).view(np.uint8)
        blob[48:64, c0:c0 + bw] = _wrap16(ueA_idx).view(np.uint8)
        blob[64:80, c0:c0 + bw] = _wrap16(ueB_idx).view(np.uint8)
        c0, _ = L["bpar"]
        blob[:, c0:c0 + 16] = (uid_c & 1).astype(np.uint8).reshape(16, 128).T
        blob[:, c0 + 16:c0 + 32] = (iid_c & 1).astype(np.uint8).reshape(16, 128).T
        c0, _ = L["idg_u"]
        blob[:, c0:c0 + 4 * Wc_u] = \
            inv_deg_grid(c, Wc_u, n_win_u, uu, deg_u_full).view(np.uint8)
        c0, _ = L["idg_i"]
        blob[:, c0:c0 + 4 * Wc_i] = \
            inv_deg_grid(c, Wc_i, n_win_i, ui, deg_i_full).view(np.uint8)
        c0, _ = L["W1"]
        for k in range(4):
            r0 = (k % 2) * 64
            cc = c0 + (k // 2) * 512
            blob[r0:r0 + 64, cc:cc + 512] = \
                np.ascontiguousarray(W1f[64 * k:64 * k + 64, :]).view(np.uint8)
        c0, _ = L["W2"]; blob[:, c0:c0 + 256] = W2f.view(np.uint8)
        c0, _ = L["WuWi"]
        blob[0:64, c0:c0 + 256] = Wuf.view(np.uint8)
        blob[64:128, c0:c0 + 256] = Wif.view(np.uint8)
        c0, _ = L["sm"]; blob[:, c0:c0 + 12] = sm.view(np.uint8)
        in_maps.append(dict(
            tsh=np.concatenate([utab_sh[c], itab_sh[c]], axis=0),
            blob=blob,
        ))
    return dict(T_u=T_u, off_u=off_u0, half_u=half_u0, T_i=T_i, off_i=off_i0,
                half_i=half_i0, Wc_u=Wc_u, Wc_i=Wc_i, Bc=Bc,
                bias_b=bias_b), in_maps


def kernel(**inputs):
    EXEC_SECONDS.clear()
    args, in_maps = host_prep(**inputs)
    nc1 = build_fused(args["T_u"], args["off_u"], args["half_u"], args["T_i"],
                      args["off_i"], args["half_i"], args["Wc_u"], args["Wc_i"],
                      args["Bc"])
    _t0 = _time.perf_counter()
    res = bass_utils.run_bass_kernel_spmd(nc1, in_maps, core_ids=list(range(N_CORES)))
    EXEC_SECONDS.append(_time.perf_counter() - _t0)
    out = np.concatenate([res.results[c]["out"][0] for c in range(N_CORES)])
    return (out + args["bias_b"]).astype(np.float32)


# revision 4
# speedup vs baseline: 2.4225x; 1.0511x over previous
"""Self-contained Trainium2 Bass kernel for nn_GCMCModel (GCMC GNN), v2.

Single fused launch:
  - bf16 embedding tables uploaded SHARDED (1/8 per core), AllGather'd on
    device over NeuronLink (replaces 8x replicated f32 upload = the old
    bottleneck: ~300MB over a ~60MB/s axon tunnel).
  - segment-sum aggregation via dma_gather + one-hot matmuls (as v1).
  - per-window inv-degree scaling + PE transpose, AllGather of the
    slot-major partial aggregates, per-batch gather, MLP, all on-core.
  - downloads only the final [1, B/8] per core.
"""

# ---- toolchain workarounds (this container's walrus supports only one
# sync-wait per instruction) -------------------------------------------------

def _apply_tile_fix():
    import concourse.mybir as mybir
    from concourse.tile import TileContext, ScopedClock
    if getattr(TileContext, "_drain_patched", False):
        return
    TileContext._drain_patched = True

    def _drain_and_barrier(self, tick_clock, wait_clock):
        nop = self.nc.sync.nop()
        wait_clock.add_sem_waits(nop.ins, ScopedClock({None: tick_clock.global_clock}))
        si = nop.ins.sync_info
        waits = list(si.on_wait) if si is not None else []
        if waits:
            si.on_wait = waits[:1]
        for w in waits[1:]:
            n2 = self.nc.sync.nop()
            n2.ins.sync_info = mybir.SyncInfo(on_wait=[w], on_update=[])
        self.nc.sync.drain()
        self.nc.all_engine_barrier()
        popped = self.nc._tile_sem_poison_stack.pop()
        assert popped is self._sem_poison
        self.nc.clear_and_free_semaphores(list(self.sems.allocated().values()))
        self.nc.all_engine_barrier()

    TileContext._drain_and_barrier = _drain_and_barrier


def _apply_bir_fix():
    import json as _json
    import concourse.bass_utils as _bu
    import concourse.bass2jax as _b2j
    if getattr(_bu, "_wait_split_patched", False):
        return
    _bu._wait_split_patched = True
    _orig = _bu.compile_bir_kernel
    _ctr = [0]

    def _split(bir_bytes):
        mod = _json.loads(bir_bytes)
        changed = False
        for fn in mod.get("functions", []):
            for blk in fn.get("blocks", []) or []:
                out = []
                for ins in blk.get("instructions", []):
                    si = ins.get("sync_info")
                    waits = (si or {}).get("on_wait") or []
                    if len(waits) > 1:
                        changed = True
                        for w in waits[:-1]:
                            _ctr[0] += 1
                            out.append({"debug": ins.get("debug", 0),
                                        "engine": ins["engine"], "ins": [],
                                        "name": f"{ins['name']}-ws{_ctr[0]}",
                                        "opcode": "NoOp", "outs": [],
                                        "sync_info": {"on_update": [],
                                                      "on_wait": [w]}})
                        si["on_wait"] = [waits[-1]]
                    out.append(ins)
                blk["instructions"] = out
        return _json.dumps(mod).encode() if changed else bir_bytes

    def _patched(bir_json, tmpdir, neff_name="file.neff"):
        if isinstance(bir_json, str):
            bir_json = bir_json.encode()
        return _orig(_split(bir_json), tmpdir, neff_name)

    _bu.compile_bir_kernel = _patched
    _b2j.compile_bir_kernel = _patched

_apply_tile_fix()
_apply_bir_fix()

import time as _time
import numpy as np
import concourse.bacc as bacc
import concourse.mybir as mybir
from concourse.tile import TileContext
from concourse import bass_utils

EXEC_SECONDS = []

N_CORES = 8
P = 128
GG = 32          # tiles per dma_gather group
UHALF = 65024    # user table gather split point (must be even)

# packed-pairs table geometry (two 64-d rows per 128-wide packed row, one
# leading zero pair-row, padded to a multiple of 8 for sharding)
N_USER, N_ITEM = 100000, 50000
NUT = -((-(N_USER // 2 + 1)) // 8) * 8     # 50008 packed user rows
NIT = -((-(N_ITEM // 2 + 1)) // 8) * 8     # 25008 packed item rows
NUS, NIS = NUT // 8, NIT // 8              # per-core shard rows
A_ROWS = UHALF // 2 + 1                    # 32513: packed rows for users < UHALF
B_BASE = A_ROWS - 1                        # 32512: B slice starts here
B_ROWS = NUT - B_BASE                      # 17496
PADB = (N_USER // 2 + 1) - B_BASE          # 17489: zero pad row inside B slice


def _build_side(n_slots, slot_of_edge, val_idx, val_par, val_half, n_cores):
    """Bin edges by (core, window, half) into a STRUCTURALLY UNIFORM tile grid:
    every core gets Wc windows x (KA half-A tiles + KB half-B tiles). Tile t:
    window = t // K, half = 0 if t % K < KA else 1, acc offset = window*128.
    Pad slots use value-idx 0 (a zero row), so they contribute nothing."""
    w = (slot_of_edge >> 7).astype(np.int64)
    n_win = (n_slots + 127) // 128
    Wc = (n_win + n_cores - 1) // n_cores
    r = (slot_of_edge & 127).astype(np.int64)
    halves = val_half if val_half is not None else np.zeros(len(w), np.int8)

    key = w * 2 + halves
    order = np.argsort(key, kind="stable")
    key_s = key[order]
    starts = np.searchsorted(key_s, np.arange(n_win * 2))
    ends = np.searchsorted(key_s, np.arange(n_win * 2) + 1)
    cnt = (ends - starts).reshape(n_win, 2)
    KA = max(1, int(np.ceil(cnt[:, 0].max() / P))) if cnt[:, 0].max() else 1
    KB = int(np.ceil(cnt[:, 1].max() / P)) if val_half is not None and cnt[:, 1].max() else 0
    K = KA + KB
    T = Wc * K

    half_tile = np.zeros(T, np.int8)
    off_tile = np.zeros(T, np.int64)
    for t in range(T):
        off_tile[t] = (t // K) * 128
        half_tile[t] = 0 if (t % K) < KA else 1

    per_core = []
    for c in range(n_cores):
        r_grid = np.zeros((P, T), np.float16)
        p_grid = np.zeros((P, T), np.uint8)
        idx_grid = np.zeros((T, P), np.int16)
        for li in range(Wc):
            win = li * n_cores + c
            if win >= n_win:
                continue
            for h, base_t, cap in ((0, li * K, KA), (1, li * K + KA, KB)):
                g0, g1 = starts[win * 2 + h], ends[win * 2 + h]
                eids = order[g0:g1]
                assert len(eids) <= cap * P
                for j, s0 in enumerate(range(0, len(eids), P)):
                    seg = eids[s0:s0 + P]
                    n = len(seg)
                    t = base_t + j
                    r_grid[:n, t] = r[seg]
                    p_grid[:n, t] = val_par[seg]
                    idx_grid[t, :n] = val_idx[seg]
        # half-B tiles gather from utab[B_BASE:]; their pad lanes (idx 0)
        # must point at a zero row INSIDE the B slice.
        for t in range(T):
            if half_tile[t] == 1:
                col = idx_grid[t]
                col[col == 0] = PADB
        per_core.append(dict(r_grid=r_grid, p_grid=p_grid, idx_grid=idx_grid,
                             half_tile=half_tile, off_tile=off_tile))
    return per_core, Wc, T


def _wrap16(idx_flat):
    """[N] int16 -> [16, N/16] (16-lane wrap; device replicates x8)."""
    n = len(idx_flat)
    assert n % 16 == 0
    return np.ascontiguousarray(idx_flat.reshape(n // 16, 16).T)




def _layout(TT, Wc_u, Wc_i, Bc):
    """Column layout (u8) of the consolidated per-core blob [128, CB]."""
    def al(x, a):
        return -(-x // a) * a
    off = {}
    c = 0
    off["rgp"] = (c, TT); c = al(c + TT, 2)
    off["ix"] = (c, 2 * TT); c += 2 * TT
    off["brow"] = (c, 2 * (Bc // 16)); c += 2 * (Bc // 16)
    off["bpar"] = (c, 32); c = al(c + 32, 4)
    off["idg_u"] = (c, 4 * Wc_u); c += 4 * Wc_u
    off["idg_i"] = (c, 4 * Wc_i); c += 4 * Wc_i
    off["W1"] = (c, 1024); c += 1024
    off["W2"] = (c, 256); c += 256
    off["WuWi"] = (c, 256); c += 256
    off["sm"] = (c, 12); c += 12
    return off, al(c, 4)

def build_fused(T_u, off_u, half_u, T_i, off_i, half_i, Wc_u, Wc_i, Bc,
                no_gather=False, no_compute=False):
    nc = bacc.Bacc()
    dt = mybir.dt
    TT = T_u + T_i
    GU = Bc // 128   # per-b gather groups per side
    RG8 = list(range(8))

    L, CB = _layout(TT, Wc_u, Wc_i, Bc)
    tsh = nc.dram_tensor("tsh", [NUS + NIS, 128], dt.float16, kind="ExternalInput")
    blob = nc.dram_tensor("blob", [128, CB], dt.uint8, kind="ExternalInput")
    out = nc.dram_tensor("out", [1, Bc], dt.float32, kind="ExternalOutput")

    CH = 512
    with TileContext(nc) as tc:
        with tc.tile_pool(name="st", bufs=1) as st, \
             tc.tile_pool(name="dram", bufs=1, space="DRAM") as dram:
            # ---- DRAM scratch + table AllGather ----
            ish_b = dram.tile([NIS, 128], dt.float16)
            ush_b = dram.tile([NUS, 128], dt.float16)
            itab = dram.tile([NIT, 128], dt.float16)
            utab = dram.tile([NUT, 128], dt.float16)
            accT_u_l = dram.tile([Wc_u * 128, 64], dt.float32)
            accT_i_l = dram.tile([Wc_i * 128, 64], dt.float32)
            accT_u_f = dram.tile([Wc_u * 128 * 8, 64], dt.float32)
            accT_i_f = dram.tile([Wc_i * 128 * 8, 64], dt.float32)
            nc.gpsimd.dma_start(ush_b[:, :], tsh[0:NUS, :])
            nc.gpsimd.dma_start(ish_b[:, :], tsh[NUS:NUS + NIS, :])
            nc.gpsimd.collective_compute(
                "AllGather", mybir.AluOpType.bypass, replica_groups=[RG8],
                ins=[ish_b.opt()], outs=[itab.opt()])
            nc.gpsimd.collective_compute(
                "AllGather", mybir.AluOpType.bypass, replica_groups=[RG8],
                ins=[ush_b.opt()], outs=[utab.opt()])

            # ---- SBUF staging (from consolidated blob) ----
            ixt = st.tile([128, TT * 8], dt.int16)
            c0, _ = L["ix"]
            for r in range(8):
                for k in range(8):
                    nc.sync.dma_start(
                        out=ixt[16 * r:16 * r + 16, k * TT:(k + 1) * TT],
                        in_=blob[16 * k:16 * k + 16,
                                 c0:c0 + 2 * TT].bitcast(dt.int16))
            c0, _ = L["rgp"]
            rgp = st.tile([P, TT], dt.uint8)
            nc.sync.dma_start(out=rgp[:], in_=blob[:, c0:c0 + TT])
            rgc = st.tile([P, TT], dt.uint8)
            pm = st.tile([P, TT], dt.uint8)
            nc.vector.tensor_scalar(out=rgc[:], in0=rgp[:], scalar1=127,
                                    scalar2=None,
                                    op0=mybir.AluOpType.bitwise_and)
            nc.vector.tensor_scalar(out=pm[:], in0=rgp[:], scalar1=127,
                                    scalar2=None,
                                    op0=mybir.AluOpType.is_gt)
            c0, _ = L["idg_u"]
            idg_ut = st.tile([P, Wc_u], dt.float32)
            nc.sync.dma_start(out=idg_ut[:],
                              in_=blob[:, c0:c0 + 4 * Wc_u].bitcast(dt.float32))
            c0, _ = L["idg_i"]
            idg_it = st.tile([P, Wc_i], dt.float32)
            nc.sync.dma_start(out=idg_it[:],
                              in_=blob[:, c0:c0 + 4 * Wc_i].bitcast(dt.float32))

            iotaGG = st.tile([128, GG, 128], dt.uint8)
            nc.gpsimd.iota(iotaGG[:, :, :], [[0, GG], [1, 128]],
                           channel_multiplier=0,
                           allow_small_or_imprecise_dtypes=True)
            iotaI = st.tile([128, 128], dt.int32)
            nc.gpsimd.iota(iotaI[:], [[1, 128]], channel_multiplier=0)
            iota32 = st.tile([128, 128], dt.float32)
            nc.vector.tensor_copy(iota32[:], iotaI[:])
            parI = st.tile([128, 1], dt.int32)
            nc.gpsimd.iota(parI[:], [[0, 1]], channel_multiplier=1)
            parF = st.tile([128, 1], dt.float32)
            nc.vector.tensor_copy(parF[:], parI[:])
            ident = st.tile([128, 128], dt.float32)
            nc.vector.tensor_tensor(out=ident[:], in0=iota32[:],
                                    in1=parF[:].to_broadcast([128, 128]),
                                    op=mybir.AluOpType.is_equal)

            acc_u = st.tile([64, Wc_u * 128], dt.float32)
            acc_i = st.tile([64, Wc_i * 128], dt.float32)
            nc.vector.memset(acc_u[:], 0.0)
            nc.vector.memset(acc_i[:], 0.0)

            # ---- phase A: aggregation ----
            with tc.tile_pool(name="g", bufs=2) as gp, \
                 tc.tile_pool(name="w", bufs=8) as wp, \
                 tc.tile_pool(name="psA", bufs=3, space="PSUM") as pp:
                for side in ("u", "i"):
                    T = T_u if side == "u" else T_i
                    base = 0 if side == "u" else T_u
                    offs = off_u if side == "u" else off_i
                    halves = half_u if side == "u" else half_i
                    acc = acc_u if side == "u" else acc_i
                    # gather groups: runs of tiles sharing a source table
                    groups = []
                    t0 = 0
                    while t0 < T:
                        t1 = t0
                        while t1 < T and t1 - t0 < GG and halves[t1] == halves[t0]:
                            t1 += 1
                        groups.append((t0, t1))
                        t0 = t1
                    oh_of = {}
                    vs_of = {}
                    for (a, b) in groups:
                        nt = b - a
                        if side == "u":
                            tab_ap = itab[:, :]
                        else:
                            tab_ap = (utab[0:A_ROWS, :] if halves[a] == 0
                                      else utab[B_BASE:NUT, :])
                        vp = gp.tile([P, GG, 128], dt.float16, tag="vp")
                        if no_gather:
                            nc.vector.memset(vp[:, :nt, :], 0.0)
                        else:
                            nc.gpsimd.dma_gather(
                                out_ap=vp[:, :nt, :], in_ap=tab_ap,
                                idxs_ap=ixt[:, (base + a) * 8:(base + b) * 8],
                                num_idxs=nt * 128, num_idxs_reg=nt * 128,
                                elem_size=128, single_packet=False)
                        oh = gp.tile([P, GG, 128], dt.float16, tag="oh")
                        nc.vector.tensor_tensor(
                            out=oh[:, :nt, :],
                            in0=rgc[:, base + a:base + b].to_broadcast([P, nt, 128]),
                            in1=iotaGG[:, :nt, :],
                            op=mybir.AluOpType.is_equal)
                        vs = gp.tile([P, GG, 64], dt.float16, tag="vs")
                        nc.vector.select(
                            out=vs[:, :nt, :],
                            mask=pm[:, base + a:base + b].to_broadcast([P, nt, 64]),
                            on_true=vp[:, :nt, 64:128],
                            on_false=vp[:, :nt, 0:64])
                        for t in range(a, b):
                            oh_of[t] = (oh, t - a)
                            vs_of[t] = (vs, t - a)
                    t = 0
                    while t < T and not no_compute:
                        o = int(offs[t])
                        K_w = 1
                        while t + K_w < T and int(offs[t + K_w]) == o:
                            K_w += 1
                        ps = pp.tile([64, 128], dt.float32, tag="ps")
                        for j in range(K_w):
                            tt = t + j
                            oh, oi = oh_of[tt]
                            vs, vi = vs_of[tt]
                            nc.tensor.matmul(ps[:], lhsT=vs[:, vi, :],
                                             rhs=oh[:, oi, :],
                                             start=(j == 0), stop=(j == K_w - 1))
                        nc.vector.tensor_add(
                            out=acc[:, o:o + 128], in0=acc[:, o:o + 128], in1=ps[:])
                        t += K_w

                # ---- device-side ue/ie gather from AllGather'd tables ----
                ident16 = st.tile([128, 128], dt.float16)
                nc.vector.tensor_copy(ident16[:], ident[:])
                cpar, _ = L["bpar"]
                pu_t = st.tile([128, 16], dt.uint8)
                pi_t = st.tile([128, 16], dt.uint8)
                nc.sync.dma_start(out=pu_t[:], in_=blob[:, cpar:cpar + 16])
                nc.sync.dma_start(out=pi_t[:], in_=blob[:, cpar + 16:cpar + 32])
                cbr, _ = L["brow"]
                bw = 2 * (Bc // 16)
                eidx = {}
                for nm, rr in (("ie", (32, 48)), ("ueA", (48, 64)),
                               ("ueB", (64, 80))):
                    ei_t = st.tile([128, Bc // 16], dt.int16, tag=f"eix{nm}")
                    for k in range(8):
                        nc.sync.dma_start(
                            out=ei_t[16 * k:16 * k + 16, :],
                            in_=blob[rr[0]:rr[1], cbr:cbr + bw].bitcast(dt.int16))
                    eidx[nm] = ei_t
                g_ie = st.tile([128, GU, 128], dt.float16)
                nc.gpsimd.dma_gather(
                    out_ap=g_ie[:, :, :], in_ap=itab[:, :], idxs_ap=eidx["ie"][:, :],
                    num_idxs=Bc, num_idxs_reg=Bc, elem_size=128, single_packet=False)
                g_uA = st.tile([128, GU, 128], dt.float16)
                nc.gpsimd.dma_gather(
                    out_ap=g_uA[:, :, :], in_ap=utab[0:A_ROWS, :],
                    idxs_ap=eidx["ueA"][:, :],
                    num_idxs=Bc, num_idxs_reg=Bc, elem_size=128, single_packet=False)
                g_uB = st.tile([128, GU, 128], dt.float16)
                nc.gpsimd.dma_gather(
                    out_ap=g_uB[:, :, :], in_ap=utab[B_BASE:NUT, :],
                    idxs_ap=eidx["ueB"][:, :],
                    num_idxs=Bc, num_idxs_reg=Bc, elem_size=128, single_packet=False)
                g_ue = st.tile([128, GU, 128], dt.float16)
                nc.vector.tensor_add(g_ue[:, :, :], g_uA[:, :, :], g_uB[:, :, :])
                se_u = st.tile([128, GU, 64], dt.float16)
                nc.vector.select(
                    out=se_u[:, :, :],
                    mask=pu_t[:, :].to_broadcast([128, GU, 64]),
                    on_true=g_ue[:, :, 64:128], on_false=g_ue[:, :, 0:64])
                se_i = st.tile([128, GU, 64], dt.float16)
                nc.vector.select(
                    out=se_i[:, :, :],
                    mask=pi_t[:, :].to_broadcast([128, GU, 64]),
                    on_true=g_ie[:, :, 64:128], on_false=g_ie[:, :, 0:64])

                # ---- inv-deg scale + transpose to slot-major, local DRAM ----
                for side in ("u", "i"):
                    Wc = Wc_u if side == "u" else Wc_i
                    acc = acc_u if side == "u" else acc_i
                    idg = idg_ut if side == "u" else idg_it
                    accT = accT_u_l if side == "u" else accT_i_l
                    for li in range(Wc):
                        pT = pp.tile([128, 64], dt.float32, tag="pt")
                        nc.tensor.transpose(pT[:], acc[:, li * 128:(li + 1) * 128],
                                            ident[0:64, 0:64])
                        sT = wp.tile([128, 64], dt.float32, tag="st")
                        nc.vector.tensor_tensor(
                            out=sT[:], in0=pT[:],
                            in1=idg[:, li:li + 1].to_broadcast([128, 64]),
                            op=mybir.AluOpType.mult)
                        nc.sync.dma_start(out=accT[li * 128:(li + 1) * 128, :],
                                          in_=sT[:])
                nc.gpsimd.collective_compute(
                    "AllGather", mybir.AluOpType.bypass, replica_groups=[RG8],
                    ins=[accT_u_l.opt()], outs=[accT_u_f.opt()])
                nc.gpsimd.collective_compute(
                    "AllGather", mybir.AluOpType.bypass, replica_groups=[RG8],
                    ins=[accT_i_l.opt()], outs=[accT_i_f.opt()])

            # ---- phase B: per-b gather + MLP ----
            with tc.tile_pool(name="w2", bufs=2) as wp2, \
                 tc.tile_pool(name="psB", bufs=1, space="PSUM") as pp2:
                gu = st.tile([64, Bc], dt.float32)
                gi = st.tile([64, Bc], dt.float32)
                cbr, _ = L["brow"]
                for side in ("u", "i"):
                    brows = (0, 16) if side == "u" else (16, 32)
                    accF = accT_u_f if side == "u" else accT_i_f
                    dst = gi if side == "u" else gu  # user-side agg -> gcn_item_h
                    browt = st.tile([128, Bc // 16], dt.int16,
                                    tag=f"brow{side}")
                    for k in range(8):
                        nc.sync.dma_start(
                            out=browt[16 * k:16 * k + 16, :],
                            in_=blob[brows[0]:brows[1],
                                     cbr:cbr + 2 * (Bc // 16)].bitcast(dt.int16))
                    gg = st.tile([128, GU, 64], dt.float32, tag=f"gg{side}")
                    nc.gpsimd.dma_gather(
                        out_ap=gg[:, :, :], in_ap=accF[:, :],
                        idxs_ap=browt[:, :], num_idxs=Bc, num_idxs_reg=Bc,
                        elem_size=64, single_packet=False)
                    for k in range(GU):
                        pG = pp2.tile([64, 128], dt.float32, tag="pg")
                        nc.tensor.transpose(pG[:], gg[:, k, :], ident[:, :])
                        nc.vector.tensor_copy(dst[:, k * 128:(k + 1) * 128], pG[:])

                ue_f = st.tile([64, Bc], dt.float32)
                ie_f = st.tile([64, Bc], dt.float32)
                for se, dst in ((se_u, ue_f), (se_i, ie_f)):
                    for k in range(GU):
                        pE = pp2.tile([64, 128], dt.float16, tag="pe")
                        nc.tensor.matmul(pE[:], lhsT=se[:, k, :], rhs=ident16[:],
                                         is_transpose=True, start=True, stop=True)
                        nc.vector.tensor_copy(dst[:, k * 128:(k + 1) * 128], pE[:])

                cW1, _ = L["W1"]
                t_W1 = st.tile([64, 4 * 128], dt.float32)
                for k in range(4):
                    r0 = (k % 2) * 64
                    cc = cW1 + (k // 2) * 512
                    nc.sync.dma_start(out=t_W1[:, 128 * k:128 * k + 128],
                                      in_=blob[r0:r0 + 64,
                                               cc:cc + 512].bitcast(dt.float32))
                cW2, _ = L["W2"]
                t_W2 = st.tile([128, 64], dt.float32)
                nc.sync.dma_start(out=t_W2[:],
                                  in_=blob[:, cW2:cW2 + 256].bitcast(dt.float32))
                cWw, _ = L["WuWi"]
                t_Wu = st.tile([64, 64], dt.float32)
                t_Wi = st.tile([64, 64], dt.float32)
                nc.sync.dma_start(out=t_Wu[:],
                                  in_=blob[0:64, cWw:cWw + 256].bitcast(dt.float32))
                nc.sync.dma_start(out=t_Wi[:],
                                  in_=blob[64:128, cWw:cWw + 256].bitcast(dt.float32))
                csm, _ = L["sm"]
                t_b1 = st.tile([128, 1], dt.float32)
                nc.sync.dma_start(out=t_b1[:],
                                  in_=blob[:, csm:csm + 4].bitcast(dt.float32))
                t_bu = st.tile([64, 1], dt.float32)
                t_bi = st.tile([64, 1], dt.float32)
                nc.sync.dma_start(out=t_bu[:],
                                  in_=blob[0:64, csm + 4:csm + 8].bitcast(dt.float32))
                nc.sync.dma_start(out=t_bi[:],
                                  in_=blob[64:128, csm + 4:csm + 8].bitcast(dt.float32))
                t_b2 = st.tile([64, 1], dt.float32)
                t_W3 = st.tile([64, 1], dt.float32)
                nc.sync.dma_start(out=t_b2[:],
                                  in_=blob[0:64, csm + 8:csm + 12].bitcast(dt.float32))
                nc.sync.dma_start(out=t_W3[:],
                                  in_=blob[64:128, csm + 8:csm + 12].bitcast(dt.float32))

                guo = st.tile([64, Bc], dt.float32)
                gio = st.tile([64, Bc], dt.float32)
                h1 = st.tile([128, Bc], dt.float32)
                h2 = st.tile([64, Bc], dt.float32)
                res = st.tile([1, Bc], dt.float32)
                for c0 in range(0, Bc, CH):
                    c1 = min(c0 + CH, Bc)
                    p1 = pp2.tile([64, CH], dt.float32, tag="p1")
                    nc.tensor.matmul(p1[:, :c1 - c0], lhsT=t_Wu[:],
                                     rhs=gu[:, c0:c1], start=True, stop=True)
                    nc.scalar.activation(guo[:, c0:c1], p1[:, :c1 - c0],
                                         mybir.ActivationFunctionType.Relu,
                                         bias=t_bu[:], scale=1.0)
                    p2 = pp2.tile([64, CH], dt.float32, tag="p2")
                    nc.tensor.matmul(p2[:, :c1 - c0], lhsT=t_Wi[:],
                                     rhs=gi[:, c0:c1], start=True, stop=True)
                    nc.scalar.activation(gio[:, c0:c1], p2[:, :c1 - c0],
                                         mybir.ActivationFunctionType.Relu,
                                         bias=t_bi[:], scale=1.0)
                    prods = []
                    for (x_, y_) in ((ue_f, ie_f), (ue_f, gio), (guo, ie_f),
                                     (guo, gio)):
                        pr = wp2.tile([64, CH], dt.float32,
                                      tag=f"pr{len(prods)}")
                        nc.vector.tensor_mul(pr[:, :c1 - c0], x_[:, c0:c1],
                                             y_[:, c0:c1])
                        prods.append(pr)
                    p3 = pp2.tile([128, CH], dt.float32, tag="p3")
                    for k in range(4):
                        nc.tensor.matmul(p3[:, :c1 - c0],
                                         lhsT=t_W1[:, 128 * k:128 * k + 128],
                                         rhs=prods[k][:, :c1 - c0],
                                         start=(k == 0), stop=(k == 3))
                    nc.scalar.activation(h1[:, c0:c1], p3[:, :c1 - c0],
                                         mybir.ActivationFunctionType.Tanh,
                                         bias=t_b1[:], scale=1.0)
                    p4 = pp2.tile([64, CH], dt.float32, tag="p4")
                    nc.tensor.matmul(p4[:, :c1 - c0], lhsT=t_W2[:],
                                     rhs=h1[:, c0:c1], start=True, stop=True)
                    nc.scalar.activation(h2[:, c0:c1], p4[:, :c1 - c0],
                                         mybir.ActivationFunctionType.Tanh,
                                         bias=t_b2[:], scale=1.0)
                    p5 = pp2.tile([1, CH], dt.float32, tag="p5")
                    nc.tensor.matmul(p5[:, :c1 - c0], lhsT=t_W3[:],
                                     rhs=h2[:, c0:c1], start=True, stop=True)
                    nc.vector.tensor_copy(res[:, c0:c1], p5[:, :c1 - c0])
                nc.sync.dma_start(out=out[:, :], in_=res[:])
    nc.compile()
    return nc


def host_prep(user_table, item_table, Wu, bu, Wi, bi, W1, b1, W2, b2, W3, b3,
              user_bias, item_bias, user_id, item_id, edge_user, edge_item):
    user_table = np.asarray(user_table, np.float32)
    item_table = np.asarray(item_table, np.float32)
    user_id = np.asarray(user_id).astype(np.int64)
    item_id = np.asarray(item_id).astype(np.int64)
    eu = np.asarray(edge_user).astype(np.int64)
    ei = np.asarray(edge_item).astype(np.int64)
    n_user, D = user_table.shape
    n_item = item_table.shape[0]
    assert (n_user, n_item) == (N_USER, N_ITEM)
    B = len(user_id)
    Bc = B // N_CORES

    # ---- host prep ----
    uu = np.unique(user_id)
    ui = np.unique(item_id)
    pos_u = np.full(n_user, -1, np.int64); pos_u[uu] = np.arange(len(uu))
    pos_i = np.full(n_item, -1, np.int64); pos_i[ui] = np.arange(len(ui))

    deg_u_full = np.bincount(eu, minlength=n_user).astype(np.float32) + 1.0
    deg_i_full = np.bincount(ei, minlength=n_item).astype(np.float32) + 1.0

    # user-side: slots over users, values = item pair-rows
    su = pos_u[eu]
    mu = su >= 0
    vi_u = ((ei[mu] >> 1) + 1).astype(np.int16)
    vp_u = (ei[mu] & 1).astype(np.float16)
    side_u, Wc_u, T_u = _build_side(len(uu), su[mu], vi_u, vp_u, None, N_CORES)

    # item-side: slots over items, values = user pair-rows (A/B halves for
    # the int16 gather-index limit)
    si = pos_i[ei]
    mi = si >= 0
    uh = (eu[mi] >= UHALF).astype(np.int8)
    relu_ = eu[mi] - uh.astype(np.int64) * UHALF
    vi_i = ((relu_ >> 1) + 1).astype(np.int16)
    # half-B indices are relative to utab[B_BASE:]: global packed row is
    # 1 + (eu >> 1); B-relative = that - B_BASE
    gb = eu[mi] >> 1
    vi_i = np.where(uh == 1, (gb + 1 - B_BASE), vi_i).astype(np.int16)
    vp_i = (eu[mi] & 1).astype(np.float16)
    side_i, Wc_i, T_i = _build_side(len(ui), si[mi], vi_i, vp_i, uh, N_CORES)

    # packed-pairs tables, bf16, padded to NCORES shards
    def pack_shards(tb, tot_rows):
        n = tb.shape[0]
        pad2 = (-n) % 2
        tbp = np.vstack([np.zeros((2, 64), np.float32), tb,
                         np.zeros((pad2, 64), np.float32)])
        packed = tbp.reshape(-1, 128)
        full = np.zeros((tot_rows, 128), np.float32)
        full[:packed.shape[0]] = packed
        return np.ascontiguousarray(full).astype(np.float16)
    itab_sh = pack_shards(item_table, NIT).reshape(N_CORES, NIS, 128)
    utab_sh = pack_shards(user_table, NUT).reshape(N_CORES, NUS, 128)

    off_u0 = side_u[0]["off_tile"]; half_u0 = side_u[0]["half_tile"]
    off_i0 = side_i[0]["off_tile"]; half_i0 = side_i[0]["half_tile"]
    for c in range(1, N_CORES):
        assert (side_u[c]["off_tile"] == off_u0).all()
        assert (side_i[c]["off_tile"] == off_i0).all()
        assert (side_u[c]["half_tile"] == half_u0).all()
        assert (side_i[c]["half_tile"] == half_i0).all()

    n_win_u = (len(uu) + 127) // 128
    n_win_i = (len(ui) + 127) // 128

    # per-b rows into the AllGather'd slot-major layout:
    # row(slot) = (w%8)*Wc*128 + (w//8)*128 + (slot&127),  w = slot>>7
    def slot_rows(slots, Wc):
        w = slots >> 7
        return ((w % N_CORES) * (Wc * 128) + (w // N_CORES) * 128
                + (slots & 127)).astype(np.int16)
    brow_u_all = slot_rows(pos_u[user_id], Wc_u)
    brow_i_all = slot_rows(pos_i[item_id], Wc_i)

    # inv-degree per slot, laid out per core as [128, Wc]
    def inv_deg_grid(c, Wc, n_win, uniq, deg_full):
        g = np.ones((128, Wc), np.float32)
        for li in range(Wc):
            w = li * N_CORES + c
            if w >= n_win:
                continue
            s0 = w * 128
            s1 = min(s0 + 128, len(uniq))
            if s1 > s0:
                g[:s1 - s0, li] = 1.0 / deg_full[uniq[s0:s1]]
        return g

    bias_b = (np.float32(np.asarray(b3).reshape(-1)[0])
              + np.asarray(user_bias)[user_id, 0]
              + np.asarray(item_bias)[item_id, 0]).astype(np.float32)

    TT = T_u + T_i
    L, CB = _layout(TT, Wc_u, Wc_i, Bc)
    W1f = np.asarray(W1, np.float32)
    W2f = np.asarray(W2, np.float32)
    Wuf = np.asarray(Wu, np.float32)
    Wif = np.asarray(Wi, np.float32)
    sm = np.zeros((128, 3), np.float32)
    sm[:, 0] = np.asarray(b1, np.float32).reshape(-1)
    sm[0:64, 1] = np.asarray(bu, np.float32).reshape(-1)
    sm[64:128, 1] = np.asarray(bi, np.float32).reshape(-1)
    sm[0:64, 2] = np.asarray(b2, np.float32).reshape(-1)
    sm[64:128, 2] = W3f = np.asarray(W3, np.float32).reshape(-1)

    in_maps = []
    for c in range(N_CORES):
        du, di = side_u[c], side_i[c]
        sl = slice(c * Bc, (c + 1) * Bc)
        blob = np.zeros((128, CB), np.uint8)
        rp = (np.concatenate([du["r_grid"], di["r_grid"]], axis=1)
              .astype(np.uint8)
              + 128 * np.concatenate([du["p_grid"], di["p_grid"]], axis=1))
        c0, _ = L["rgp"]; blob[:, c0:c0 + TT] = rp
        ix_w = _wrap16(np.concatenate([du["idx_grid"].reshape(-1),
                                       di["idx_grid"].reshape(-1)]))
        c0, _ = L["ix"]
        for k in range(8):
            blob[16 * k:16 * k + 16, c0:c0 + 2 * TT] = \
                ix_w[:, k * TT:(k + 1) * TT].copy().view(np.uint8)
        c0, _ = L["brow"]
        bw = 2 * (Bc // 16)
        blob[0:16, c0:c0 + bw] = _wrap16(brow_u_all[sl]).view(np.uint8)
        blob[16:32, c0:c0 + bw] = _wrap16(brow_i_all[sl]).view(np.uint8)
        uid_c = user_id[sl]; iid_c = item_id[sl]
        ie_idx = (1 + (iid_c >> 1)).astype(np.int16)
        ueA_idx = np.where(uid_c < UHALF, 1 + (uid_c >> 1), 0).astype(np.int16)
        ueB_idx = np.where(uid_c >= UHALF, 1 + (uid_c >> 1) - B_BASE,
                           PADB).astype(np.int16)
        blob[32:48, c0:c0 + bw] = _wrap16(ie_idx).view(np.uint8)
        blob[48:64, c0:c0 + bw] = _wrap16(ueA_idx).view(np.uint8)
        blob[64:80, c0:c0 + bw] = _wrap16(ueB_idx).view(np.uint8)
        c0, _ = L["bpar"]
        blob[:, c0:c0 + 16] = (uid_c & 1).astype(np.uint8).reshape(16, 128).T
        blob[:, c0 + 16:c0 + 32] = (iid_c & 1).astype(np.uint8).reshape(16, 128).T
        c0, _ = L["idg_u"]
        blob[:, c0:c0 + 4 * Wc_u] = \
            inv_deg_grid(c, Wc_u, n_win_u, uu, deg_u_full).view(np.uint8)
        c0, _ = L["idg_i"]
        blob[:, c0:c0 + 4 * Wc_i] = \
            inv_deg_grid(c, Wc_i, n_win_i, ui, deg_i_full).view(np.uint8)
        c0, _ = L["W1"]
        for k in range(4):
            r0 = (k % 2) * 64
            cc = c0 + (k // 2) * 512
            blob[r0:r0 + 64, cc:cc + 512] = \
                np.ascontiguousarray(W1f[64 * k:64 * k + 64, :]).view(np.uint8)
        c0, _ = L["W2"]; blob[:, c0:c0 + 256] = W2f.view(np.uint8)
        c0, _ = L["WuWi"]
        blob[0:64, c0:c0 + 256] = Wuf.view(np.uint8)
        blob[64:128, c0:c0 + 256] = Wif.view(np.uint8)
        c0, _ = L["sm"]; blob[:, c0:c0 + 12] = sm.view(np.uint8)
        in_maps.append(dict(
            tsh=np.concatenate([utab_sh[c], itab_sh[c]], axis=0),
            blob=blob,
        ))
    return dict(T_u=T_u, off_u=off_u0, half_u=half_u0, T_i=T_i, off_i=off_i0,
                half_i=half_i0, Wc_u=Wc_u, Wc_i=Wc_i, Bc=Bc,
                bias_b=bias_b), in_maps




# ---- cached launch path -----------------------------------------------------
# run_bass_kernel_spmd re-traces a fresh jax.jit(shard_map(...)) on every call
# (only the NEFF is cached), paying XLA trace+compile inside the timed launch.
# Cache the built Bass module AND its jitted executable across kernel() calls;
# the warm launch is then transfer + execute + fetch only.
_BUILD_CACHE = {}


def _get_runner(sig, build_fn):
    if sig in _BUILD_CACHE:
        return _BUILD_CACHE[sig]
    import jax
    import concourse.mybir as _mybir
    from concourse import bass2jax
    nc = build_fn()
    bass2jax.install_neuronx_cc_hook()
    partition_name = (nc.partition_id_tensor.name
                      if nc.partition_id_tensor else None)
    in_names, out_names, out_avals, zero_shapes = [], [], [], []
    for alloc in nc.m.functions[0].allocations:
        if not isinstance(alloc, _mybir.MemoryLocationSet):
            continue
        name = alloc.memorylocations[0].name
        if alloc.kind == "ExternalInput":
            if name != partition_name:
                in_names.append(name)
        elif alloc.kind == "ExternalOutput":
            shape = tuple(alloc.tensor_shape)
            dtype = _mybir.dt.np(alloc.dtype)
            out_names.append(name)
            out_avals.append(jax.core.ShapedArray(shape, dtype))
            zero_shapes.append((shape, dtype))
    n_params = len(in_names)
    all_names = list(in_names) + list(out_names)
    if partition_name is not None:
        all_names.append(partition_name)
    donate = tuple(range(n_params, n_params + len(out_names)))

    def _body(*args):
        operands = list(args)
        if partition_name is not None:
            operands.append(bass2jax.partition_id_tensor())
        outs = bass2jax._bass_exec_p.bind(
            *operands,
            out_avals=tuple(out_avals),
            in_names=tuple(all_names),
            out_names=tuple(out_names),
            lowering_input_output_aliases=(),
            sim_require_finite=True,
            sim_require_nnan=True,
            nc=nc,
        )
        return tuple(outs)

    devices = jax.devices()[:N_CORES]
    mesh = bass2jax.Mesh(np.asarray(devices), ("core",))
    specs = (bass2jax.PartitionSpec("core"),)
    sharded = jax.jit(
        bass2jax.shard_map(
            _body, mesh=mesh,
            in_specs=specs * (n_params + len(out_names)),
            out_specs=specs * len(out_names),
            check_rep=False),
        donate_argnums=donate, keep_unused=True)
    dbg = None
    if nc.dbg_addr is not None:
        dbg = nc.dbg_addr.name

    def run(in_maps):
        maps = in_maps
        if dbg is not None:
            maps = [{**m, dbg: np.zeros((1, 2), np.uint32)} for m in maps]
        concat_in = [np.concatenate([m[name] for m in maps], axis=0)
                     for name in in_names]
        concat_zeros = [np.zeros((N_CORES * s[0], *s[1:]), d)
                        for (s, d) in zero_shapes]
        out_arrs = sharded(*concat_in, *concat_zeros)
        return [
            {name: np.asarray(out_arrs[i]).reshape(
                N_CORES, *zero_shapes[i][0])[c]
             for i, name in enumerate(out_names)}
            for c in range(N_CORES)
        ]

    _BUILD_CACHE[sig] = run
    return run


def kernel(**inputs):
    EXEC_SECONDS.clear()
    args, in_maps = host_prep(**inputs)
    sig = (args["T_u"], args["T_i"], args["Wc_u"], args["Wc_i"], args["Bc"],
           tuple(args["off_u"]), tuple(args["half_u"]),
           tuple(args["off_i"]), tuple(args["half_i"]))
    run = _get_runner(sig, lambda: build_fused(
        args["T_u"], args["off_u"], args["half_u"], args["T_i"],
        args["off_i"], args["half_i"], args["Wc_u"], args["Wc_i"],
        args["Bc"]))
    _t0 = _time.perf_counter()
    results = run(in_maps)
    EXEC_SECONDS.append(_time.perf_counter() - _t0)
    out = np.concatenate([results[c]["out"][0] for c in range(N_CORES)])
    return (out + args["bias_b"]).astype(np.float32)
